# revision 1
# baseline (speedup 1.0000x reference)
"""Trainium2 Bass kernel for AttentionNet (conv frontend + MHA + readout).

Strategy: pure data-parallel over batch (64 samples -> 8 cores x 8), with an
AllReduce for the BatchNorm batch statistics. All heavy matmuls in fp32r.

Per-core pipeline:
  conv1d (im2col matmul, K=52) -> maxpool(4) + BN partial stats (sums come
  from a host-precomputed im2col column-sum via one tiny matmul; sumsq via a
  fused DVE tensor_tensor_reduce)
  -> AllReduce stats -> BN scale+ReLU -> head-pair QKV projections (M=128)
  -> scores^T with the relative-position bias folded in as a rank-48
     factorization carried in constant contraction rows (even heads keep
     data in partitions 0:64 + consts 64:112; odd heads consts 16:64 + data
     64:128, so PSUM evictions never shift partitions)
  -> exp(s/8) over two PSUM banks per ACT op
  -> unnormalized attn @ [V | ones] (row-sums duplicated across 64
     partitions = free reciprocal broadcast) -> normalize -> Wm + ReLU with
  accum_out readout-sum -> standardize -> Wo.
"""

import sys

for p in ("/opt/trn_rl_repo", "/opt/pypackages"):
    if p not in sys.path:
        sys.path.insert(0, p)

import numpy as np

import concourse.bass as bass
import concourse.bacc as bacc
import concourse.tile as tile
import concourse.mybir as mybir
from concourse import bass_utils

F32 = mybir.dt.float32
F32R = mybir.dt.float32r
AF = mybir.ActivationFunctionType
AX = mybir.AxisListType
ALU = mybir.AluOpType

N_CORES = 8
B, L, C_IN = 64, 2000, 4
F, KW, PAD = 256, 13, 6
POOL = 4
H, DH = 8, 64
DM, NCLS = 100, 2
NOUT = L // POOL  # 500
BPC = B // N_CORES  # 8 samples per core
LP = L + 2 * PAD  # 2012
KC = F // 128  # 2 contraction chunks of 128
MSZ = 125  # NOUT split into 4 partition chunks of 125
RNK = 48  # rank of the bias factorization folded into the scores matmul
BN_N = float(B * L)  # batchnorm reduction count


def _svd_bias_factors():
    """Rank-RNK factorization of -8*bias (added to raw qk before the 1/8
    exp scale). scoresT += ut.T @ wf; each returned block is [64, NOUT] with
    rows RNK:64 zero."""
    idx = np.arange(NOUT)
    target = -8.0 * (4.0 * np.abs(idx[:, None] - idx[None, :]) + 3.0) / (L - 1)
    U, S, Vt = np.linalg.svd(target)
    uf = (U[:, :RNK] * np.sqrt(S[:RNK])).astype(np.float32)  # [NOUT, RNK]
    wf = (np.sqrt(S[:RNK])[:, None] * Vt[:RNK]).astype(np.float32)
    ut = np.zeros((64, NOUT), np.float32)
    wz = np.zeros((64, NOUT), np.float32)
    ut[:RNK] = uf.T
    wz[:RNK] = wf
    return ut, wz


_SVD_UT, _SVD_WF = _svd_bias_factors()


def _build_program():
    nc = bacc.Bacc("TRN2", target_bir_lowering=False, debug=False,
                   num_devices=N_CORES)
    dram = {}

    def din(name, shape, dt=F32R):
        dram[name] = nc.dram_tensor(name, list(shape), dt,
                                    kind="ExternalInput").ap()
        return dram[name]

    din("xpad", [BPC, C_IN, LP])      # transposed+padded x shard
    din("imsum", [C_IN * KW, 1])      # host im2col column sums (core total)
    din("gram", [C_IN * KW, C_IN * KW])  # host im2col Gram matrix (core total)
    din("wc", [C_IN * KW, F])         # conv lhsT (52, 256)
    din("wq2", [128, KC * 4 * 128])   # [p, kc*512 + pr*128 + (head-in-pair d)]
    din("wk2", [128, KC * 4 * 128])
    din("wv", [128, KC * H * DH])     # [p, kc*512 + h*64 + d]
    din("wm", [128, 4 * DM])          # [p, c*100 + j]
    din("wo", [DM + 1, NCLS])         # [Wo; bo]
    din("svd_e", [64, NOUT])          # even-head const rows: [WF; zeros]
    din("svd_eu", [64, NOUT])         # even-head const rows: [UT; zeros]
    din("svd_o", [64, NOUT])          # odd-head const rows: [zeros; WF]
    din("svd_ou", [64, NOUT])         # odd-head const rows: [zeros; UT]
    din("bqk", [128, H], F32)         # pair-layout q/k biases
    din("bm_eff", [DM], F32)          # bm + Wm^T contribution of bv
    din("gam2", [128, 2], F32)
    din("bet2", [128, 2], F32)
    out = nc.dram_tensor("out", [BPC, NCLS], F32, kind="ExternalOutput").ap()

    with tile.TileContext(nc) as tc:
        _emit(tc, dram, out)
    nc.compile()
    return nc


def _emit(tc, d, out):
    nc = tc.nc
    from contextlib import ExitStack

    ctx = ExitStack()
    with ctx:
        cst = ctx.enter_context(tc.tile_pool(name="cst", bufs=1))

        # ---- constants into SBUF ----
        wc_sb = cst.tile([C_IN * KW, F], F32R)
        nc.sync.dma_start(wc_sb[:], d["wc"][:])
        ims_sb = cst.tile([C_IN * KW, 1], F32R)
        nc.sync.dma_start(ims_sb[:], d["imsum"][:])
        gram_sb = cst.tile([C_IN * KW, C_IN * KW], F32R)
        nc.sync.dma_start(gram_sb[:], d["gram"][:])
        gam_sb = cst.tile([128, 2], F32)
        nc.sync.dma_start(gam_sb[:], d["gam2"][:])
        bet_sb = cst.tile([128, 2], F32)
        nc.sync.dma_start(bet_sb[:], d["bet2"][:])

        featn = cst.tile([DM, BPC], F32)
        ones_k = cst.tile([DM, 1], F32)
        nc.gpsimd.memset(ones_k[:], 1.0)
        ones52 = cst.tile([C_IN * KW, 1], F32)
        nc.gpsimd.memset(ones52[:], 1.0)
        ones_p = cst.tile([1, DM], F32)
        nc.gpsimd.memset(ones_p[:], 1.0)
        epst = cst.tile([128, 1], F32)
        nc.gpsimd.memset(epst[:], 1e-5)
        ones_v512 = cst.tile([128, H * DH], F32)
        nc.gpsimd.memset(ones_v512[:], 1.0)
        ones_fc = cst.tile([DM + 1, BPC], F32)
        nc.gpsimd.memset(ones_fc[:], 1.0)
        bnr = cst.tile([1, 2 * F], F32)

        with tc.tile_pool(name="imp", bufs=4) as imp, \
             tc.tile_pool(name="ymp", bufs=8) as ymp, \
             tc.tile_pool(name="htp", bufs=4) as htp, \
             tc.tile_pool(name="vtp", bufs=8) as vtp, \
             tc.tile_pool(name="ptp", bufs=6) as ptp, \
             tc.tile_pool(name="rbp", bufs=3) as rbp, \
             tc.tile_pool(name="ocp", bufs=8) as ocp, \
             tc.tile_pool(name="msp", bufs=2) as msp, \
             tc.tile_pool(name="vmp", bufs=1, space="PSUM") as vmp, \
             tc.tile_pool(name="qkp", bufs=1, space="PSUM") as qkp, \
             tc.tile_pool(name="scp", bufs=2, space="PSUM") as scp, \
             tc.tile_pool(name="cps", bufs=1, space="PSUM") as cps, \
             tc.tile_pool(name="opp", bufs=1, space="PSUM") as opp:
            # ---- BN stats from host im2col aggregates (row layout) ----
            ysr = cps.tile([1, F], F32, tag="cv", name="ysr")
            nc.tensor.matmul(ysr[:], ims_sb[:], wc_sb[:], start=True, stop=True)
            nc.vector.tensor_copy(bnr[:, 0:F], ysr[:])
            t1 = cps.tile([C_IN * KW, F], F32, tag="cv", name="t1")
            nc.tensor.matmul(t1[:], gram_sb[:], wc_sb[:], start=True, stop=True)
            pg = cst.tile([C_IN * KW, F], F32, name="pg")
            nc.vector.tensor_mul(pg[:], t1[:], wc_sb[:].bitcast(F32))
            sqr = cps.tile([1, F], F32, tag="cv", name="sqr")
            nc.tensor.matmul(sqr[:], ones52[:], pg[:], start=True, stop=True)
            nc.vector.tensor_copy(bnr[:, F:2 * F], sqr[:])

            # BN AllReduce + scale/bias, entirely on gpsimd/ACT so the DVE
            # stream is never head-of-line blocked waiting on the collective
            with tc.tile_pool(name="drp", bufs=1, space="DRAM") as drp:
                bn_in = drp.tile([1, 2 * F], F32)
                bn_out = drp.tile([1, 2 * F], F32)
                nc.scalar.dma_start(bn_in[:], bnr[:])
                nc.gpsimd.collective_compute(
                    "AllReduce", ALU.add,
                    replica_groups=[list(range(N_CORES))],
                    ins=[bn_in[:].opt()], outs=[bn_out[:].opt()])
                stg = cst.tile([1, 2 * F], F32)
                nc.scalar.dma_start(stg[:], bn_out[:])

            # transpose summed stats to column form once, finish in [128,2]
            stc = cst.tile([128, 4], F32)  # cols: sum0 sum1 sq0 sq1
            for q in range(4):
                nc.scalar.dma_start(stc[:, q:q + 1],
                                    stg[:, q * 128:(q + 1) * 128])
            mu = cst.tile([128, 2], F32)
            nc.gpsimd.tensor_scalar_mul(mu[:], stc[:, 0:2], 1.0 / BN_N)
            var = cst.tile([128, 2], F32)
            nc.gpsimd.tensor_scalar_mul(var[:], stc[:, 2:4], 1.0 / BN_N)
            musq = cst.tile([128, 2], F32)
            nc.gpsimd.tensor_mul(musq[:], mu[:], mu[:])
            nc.gpsimd.tensor_sub(var[:], var[:], musq[:])
            sdc = cst.tile([128, 2], F32)
            nc.scalar.activation(sdc[:], var[:], AF.Sqrt, bias=epst[:])
            rsdc = cst.tile([128, 2], F32)
            nc.vector.reciprocal(rsdc[:], sdc[:])
            scl = cst.tile([128, 2], F32)
            nc.gpsimd.tensor_mul(scl[:], gam_sb[:], rsdc[:])
            bia = cst.tile([128, 2], F32)
            nc.gpsimd.tensor_mul(bia[:], mu[:], scl[:])
            nc.gpsimd.tensor_sub(bia[:], bet_sb[:], bia[:])

            # ---- conv emission helpers (woven into the attention loop) ----
            imts, yms = [None] * BPC, [[None, None] for _ in range(BPC)]

            def conv_dma(s):
                imt = imp.tile([C_IN * KW, L], F32R, tag="imt", name="imt")
                xs = d["xpad"][s]  # [4, 2012]
                nc.sync.dma_start(
                    imt[:],
                    bass.AP(xs.tensor, xs.offset, [[LP, C_IN], [1, KW], [1, L]]))
                imts[s] = imt

            def conv_chunk(s, idx, pool=None):
                fh, c = idx // 4, idx % 4
                if c == 0:
                    yms[s][fh] = ymp.tile([128, NOUT], F32, tag="ym", name="ym")
                pl, tg = pool if pool is not None else (cps, "cv")
                ps = pl.tile([128, NOUT], F32, tag=tg, name="cvp")
                nc.tensor.matmul(
                    ps[:], wc_sb[:, fh * 128:(fh + 1) * 128],
                    imts[s][:, c * NOUT:(c + 1) * NOUT], start=True, stop=True)
                nc.vector.reduce_max(
                    yms[s][fh][:, c * 125:(c + 1) * 125],
                    ps[:].rearrange("p (a b) -> p a b", b=POOL), axis=AX.X)

            # prologue: sample 0 conv + weight loads
            conv_dma(0)
            conv_dma(1)
            # prologue: rotate across three otherwise-idle PSUM slots
            _pro = [(cps, "cv"), (opp, "op"), (vmp, "vm")]
            for idx in range(8):
                conv_chunk(0, idx, _pro[idx % 3])

            wq_sb = cst.tile([128, KC * 4 * 128], F32R)
            nc.sync.dma_start(wq_sb[:], d["wq2"][:])
            wk_sb = cst.tile([128, KC * 4 * 128], F32R)
            nc.sync.dma_start(wk_sb[:], d["wk2"][:])
            wv_sb = cst.tile([128, KC * H * DH], F32R)
            nc.sync.dma_start(wv_sb[:], d["wv"][:])
            wm_sb = cst.tile([128, 4 * DM], F32R)
            nc.sync.dma_start(wm_sb[:], d["wm"][:])
            wo_sb = cst.tile([DM + 1, NCLS], F32R)
            nc.sync.dma_start(wo_sb[:], d["wo"][:])
            bqk_sb = cst.tile([128, H], F32)
            nc.sync.dma_start(bqk_sb[:], d["bqk"][:])
            bm_sb = cst.tile([DM, 1], F32)
            nc.sync.dma_start(bm_sb[:], d["bm_eff"][:])
            qte, qto, kte, kto = [], [], [], []
            for i in range(2):
                te = cst.tile([128, NOUT], F32R, name=f"qte{i}")
                nc.sync.dma_start(te[64:128, :], d["svd_e"][:])
                qte.append(te)
                to = cst.tile([128, NOUT], F32R, name=f"qto{i}")
                nc.sync.dma_start(to[0:64, :], d["svd_o"][:])
                qto.append(to)
                ke = cst.tile([128, NOUT], F32R, name=f"kte{i}")
                nc.sync.dma_start(ke[64:128, :], d["svd_eu"][:])
                kte.append(ke)
                ko = cst.tile([128, NOUT], F32R, name=f"kto{i}")
                nc.sync.dma_start(ko[0:64, :], d["svd_ou"][:])
                kto.append(ko)

            # ---- main loop: attention(s) with conv(s+1) woven in ----
            for s in range(BPC):
                if s + 2 < BPC:
                    conv_dma(s + 2)
                ht = []
                for fh in range(2):
                    th = htp.tile([128, NOUT], F32, tag="th", name="th")
                    nc.gpsimd.tensor_scalar(
                        th[:], yms[s][fh][:],
                        scl[:, fh:fh + 1], bia[:, fh:fh + 1],
                        op0=ALU.mult, op1=ALU.add)
                    t = htp.tile([128, NOUT], F32R, tag="ht", name="ht")
                    nc.gpsimd.tensor_scalar_max(t[:], th[:], 0.0)
                    ht.append(t)

                # V for all heads, layout [m, 8*(64 v | 64 ones)]
                vts = []
                for mc in range(4):
                    vt = vtp.tile([MSZ, H * 128], F32R, tag="vt", name="vt")
                    nc.gpsimd.tensor_copy(
                        vt[:].rearrange("p (h x) -> p h x", x=128)[:, :, DH:128],
                        ones_v512[0:MSZ, :].rearrange("p (h x) -> p h x", x=DH))
                    vp = vmp.tile([MSZ, H * DH], F32, tag="vm", name="vp",
                                  padded_shape=[128, 512])
                    m0 = mc * MSZ
                    for kc in range(KC):
                        nc.tensor.matmul(
                            vp[:], ht[kc][:, m0:m0 + MSZ],
                            wv_sb[:, kc * 512:(kc + 1) * 512],
                            start=(kc == 0), stop=(kc == KC - 1))
                    nc.vector.tensor_copy(
                        vt[:].rearrange("p (h x) -> p h x", x=128)[:, :, 0:DH],
                        vp[:].rearrange("p (h x) -> p h x", x=DH))
                    vts.append(vt)

                ocs = [ocp.tile([128, NOUT], F32R, tag="oc", name="oc")
                       for _ in range(4)]
                for pr in range(4):
                    bb = pr % 2
                    pq = qkp.tile([128, NOUT], F32, tag="pqk", name="pq")
                    for kc in range(KC):
                        nc.tensor.matmul(
                            pq[:], wq_sb[:, kc * 512 + pr * 128:
                                         kc * 512 + (pr + 1) * 128],
                            ht[kc][:], start=(kc == 0), stop=(kc == KC - 1))
                    # evictions: even head -> rows 0:64, odd -> rows 64:128
                    nc.scalar.activation(
                        qte[bb][0:64, :], pq[0:64, :], AF.Identity,
                        bias=bqk_sb[0:64, 2 * pr:2 * pr + 1])
                    nc.vector.tensor_scalar_add(
                        qto[bb][64:128, :], pq[64:128, :],
                        bqk_sb[64:128, 2 * pr:2 * pr + 1])
                    pk = qkp.tile([128, NOUT], F32, tag="pqk", name="pk")
                    for kc in range(KC):
                        nc.tensor.matmul(
                            pk[:], wk_sb[:, kc * 512 + pr * 128:
                                         kc * 512 + (pr + 1) * 128],
                            ht[kc][:], start=(kc == 0), stop=(kc == KC - 1))
                    nc.vector.tensor_scalar_add(
                        kte[bb][0:64, :], pk[0:64, :],
                        bqk_sb[0:64, 2 * pr + 1:2 * pr + 2])
                    nc.scalar.activation(
                        kto[bb][64:128, :], pk[64:128, :], AF.Identity,
                        bias=bqk_sb[64:128, 2 * pr + 1:2 * pr + 2])

                    for h in (2 * pr, 2 * pr + 1):
                        qt_t = qte[bb] if h % 2 == 0 else qto[bb]
                        kt_t = kte[bb] if h % 2 == 0 else kto[bb]
                        op = opp.tile([128, NOUT], F32, tag="op", name="op")
                        for half in range(2):
                            sc = scp.tile([MSZ, 1024], F32, tag="sc", name="sc",
                                          padded_shape=[128, 1024])
                            pt = ptp.tile([MSZ, 1024], F32R, tag="pt", name="pt")
                            for j in range(2):
                                m0 = (half * 2 + j) * MSZ
                                nc.tensor.matmul(
                                    sc[:, j * 512:j * 512 + NOUT],
                                    kt_t[:, m0:m0 + MSZ], qt_t[:],
                                    start=True, stop=True)
                            nc.scalar.activation(
                                pt[:].rearrange("p (b x) -> p b x", b=2)[:, :, 0:NOUT],
                                sc[:].rearrange("p (b x) -> p b x", b=2)[:, :, 0:NOUT],
                                AF.Exp, scale=1.0 / 8.0)
                            for j in range(2):
                                mc = half * 2 + j
                                nc.tensor.matmul(
                                    op[:], vts[mc][:, h * 128:(h + 1) * 128],
                                    pt[:, j * 512:j * 512 + NOUT],
                                    start=(mc == 0), stop=(mc == 3))
                        rb = rbp.tile([128, NOUT], F32, tag="rb", name="rb")
                        nc.vector.reciprocal_approx_fast(rb[:], op[:])
                        nc.vector.tensor_mul(
                            ocs[h // 2][(h % 2) * 64:(h % 2) * 64 + 64, :],
                            op[0:64, :], rb[64:128, :])
                        # weave one conv chunk of the next sample per head
                        if s + 1 < BPC:
                            conv_chunk(s + 1, h)

                mp = scp.tile([DM, NOUT], F32, tag="sc", name="mp",
                              padded_shape=[128, 1024])
                for c in range(4):
                    nc.tensor.matmul(mp[:], wm_sb[:, c * DM:(c + 1) * DM],
                                     ocs[c][:], start=(c == 0), stop=(c == 3))
                ms = msp.tile([DM, NOUT], F32)
                nc.scalar.activation(ms[:], mp[:], AF.Relu, bias=bm_sb[:],
                                     accum_out=featn[:, s:s + 1])

        # ---- final: standardize + Wo ----
        with tc.tile_pool(name="fsp", bufs=1) as fsp, \
             tc.tile_pool(name="fpp", bufs=1, space="PSUM") as fpp:
            fsq = fsp.tile([DM, BPC], F32)
            nc.scalar.activation(fsq[:], featn[:], AF.Square)
            cs = fpp.tile([1, 2 * BPC], F32, tag="cs")
            nc.tensor.matmul(cs[:, 0:BPC], ones_k[:], featn[:], start=True,
                             stop=True)
            nc.tensor.matmul(cs[:, BPC:2 * BPC], ones_k[:], fsq[:], start=True,
                             stop=True)
            st = fsp.tile([1, 4 * BPC], F32)
            nc.vector.tensor_scalar_mul(st[:, 0:BPC], cs[:, 0:BPC], 1.0 / DM)
            nc.vector.tensor_scalar_mul(st[:, BPC:2 * BPC], cs[:, BPC:2 * BPC],
                                        1.0 / DM)
            nc.vector.tensor_mul(st[:, 2 * BPC:3 * BPC], st[:, 0:BPC],
                                 st[:, 0:BPC])
            nc.vector.tensor_sub(st[:, 3 * BPC:4 * BPC], st[:, BPC:2 * BPC],
                                 st[:, 2 * BPC:3 * BPC])
            sdt = fsp.tile([1, BPC], F32, tag="sdt")
            nc.scalar.activation(sdt[:], st[:, 3 * BPC:4 * BPC], AF.Sqrt)
            nc.vector.tensor_scalar_add(sdt[:], sdt[:], 1e-6)
            rsd = fsp.tile([1, BPC], F32, tag="rsd")
            nc.vector.reciprocal(rsd[:], sdt[:])
            bcm = fpp.tile([DM, BPC], F32, tag="bcm")
            nc.tensor.matmul(bcm[:], ones_p[:], st[:, 0:BPC], start=True,
                             stop=True)
            bcr = fpp.tile([DM, BPC], F32, tag="bcr")
            nc.tensor.matmul(bcr[:], ones_p[:], rsd[:], start=True, stop=True)
            fc = fsp.tile([DM, BPC], F32, tag="fc")
            nc.vector.tensor_sub(fc[:], featn[:], bcm[:])
            fcn = fsp.tile([DM + 1, BPC], F32R, tag="fcn")
            nc.vector.tensor_copy(fcn[:], ones_fc[:])
            nc.vector.tensor_mul(fcn[0:DM, :], fc[:], bcr[:])
            fo = fpp.tile([BPC, NCLS], F32, tag="fo")
            nc.tensor.matmul(fo[:], fcn[:], wo_sb[:], start=True, stop=True)
            osb = fsp.tile([BPC, NCLS], F32, tag="osb")
            nc.vector.tensor_copy(osb[:], fo[:])
            nc.sync.dma_start(out[:], osb[:])


_NC_CACHE = None


def _get_program():
    global _NC_CACHE
    if _NC_CACHE is None:
        _NC_CACHE = _build_program()
    return _NC_CACHE


def _prep_inputs(x, conv_w, bn_gamma, bn_beta, Wq, bq, Wk, bk, Wv, bv, Wm, bm,
                 Wo, bo):
    f32 = np.float32
    x = np.asarray(x, f32)
    xpad = np.zeros((B, C_IN, LP), f32)
    xpad[:, :, PAD:PAD + L] = np.transpose(x, (0, 2, 1))
    wc = np.ascontiguousarray(
        np.transpose(np.asarray(conv_w, f32), (1, 2, 0)).reshape(C_IN * KW, F))

    def pair_layout(W):  # [H, F, DH] -> [128, kc*512 + pr*128 + (64h0|64h1)]
        W = np.asarray(W, f32).reshape(4, 2, KC, 128, DH)  # pr, hp, kc, p, d
        o = np.transpose(W, (3, 2, 0, 1, 4))  # p, kc, pr, hp, d
        return np.ascontiguousarray(o.reshape(128, KC * 4 * 128))

    wq2, wk2 = pair_layout(Wq), pair_layout(Wk)
    wvh = np.ascontiguousarray(
        np.transpose(np.asarray(Wv, f32).reshape(H, KC, 128, DH),
                     (2, 1, 0, 3)).reshape(128, KC * H * DH))
    wmh = np.ascontiguousarray(
        np.transpose(np.asarray(Wm, f32).reshape(4, 128, DM), (1, 0, 2))
        .reshape(128, 4 * DM))
    woh = np.concatenate([np.asarray(Wo, f32),
                          np.asarray(bo, f32)[None, :]], axis=0)
    # pair-layout biases: col 2pr = [bq[2pr]; bq[2pr+1]], col 2pr+1 for bk
    bq_, bk_ = np.asarray(bq, f32), np.asarray(bk, f32)
    bqkh = np.zeros((128, H), f32)
    for pr in range(4):
        bqkh[0:64, 2 * pr] = bq_[2 * pr]
        bqkh[64:128, 2 * pr] = bq_[2 * pr + 1]
        bqkh[0:64, 2 * pr + 1] = bk_[2 * pr]
        bqkh[64:128, 2 * pr + 1] = bk_[2 * pr + 1]
    bv_f = np.asarray(bv, f32).reshape(H * DH)
    bmh = np.asarray(bm, f32) + bv_f @ np.asarray(Wm, f32)
    gam2 = np.ascontiguousarray(np.asarray(bn_gamma, f32).reshape(2, 128).T)
    bet2 = np.ascontiguousarray(np.asarray(bn_beta, f32).reshape(2, 128).T)

    # odd-head const layouts: [16 zero pad; 48 factor rows]
    svd_o = np.zeros((64, NOUT), f32)
    svd_o[16:64] = _SVD_WF[0:RNK]
    svd_ou = np.zeros((64, NOUT), f32)
    svd_ou[16:64] = _SVD_UT[0:RNK]

    # per-core im2col aggregates: column sums [52,1] and Gram [52,52]
    win = np.lib.stride_tricks.sliding_window_view(xpad, KW, axis=2)  # B,C,L,K
    im_all = np.ascontiguousarray(
        win.transpose(0, 1, 3, 2).reshape(B, C_IN * KW, L))

    shared = dict(wc=wc, wq2=wq2, wk2=wk2, wv=wvh, wm=wmh, wo=woh, bqk=bqkh,
                  bm_eff=bmh, gam2=gam2, bet2=bet2,
                  svd_e=_SVD_WF, svd_eu=_SVD_UT, svd_o=svd_o, svd_ou=svd_ou)
    in_maps = []
    for c in range(N_CORES):
        m = dict(shared)
        m["xpad"] = np.ascontiguousarray(xpad[c * BPC:(c + 1) * BPC])
        a = im_all[c * BPC:(c + 1) * BPC].transpose(1, 0, 2).reshape(
            C_IN * KW, BPC * L)
        m["imsum"] = a.sum(axis=1, dtype=np.float64).astype(f32).reshape(-1, 1)
        m["gram"] = (a.astype(np.float64) @ a.astype(np.float64).T).astype(f32)
        in_maps.append(m)
    return in_maps


def kernel(**inputs):
    in_maps = _prep_inputs(**inputs)
    nc = _get_program()
    res = bass_utils.run_bass_kernel_spmd(nc, in_maps, list(range(N_CORES)))
    return np.concatenate([res.results[i]["out"] for i in range(N_CORES)],
                          axis=0).astype(np.float32)



# revision 32
# speedup vs baseline: 405.4004x; 405.4004x over previous
"""Trainium2 Bass kernel for AttentionNet (conv frontend + MHA + readout).

Strategy: pure data-parallel over batch (64 samples -> 8 cores x 8). BatchNorm
batch statistics are computed exactly on the host from the im2col column-sum /
Gram aggregates (the affine scale folds into the conv weights, which commutes
with maxpool+ReLU because scale > 0), so there is no on-device collective and
no BN-stats prologue at all. All heavy matmuls in fp32r.

Per-core pipeline:
  conv1d (im2col matmul, K=52) -> maxpool(4) -> fused bias+ReLU (ACT)
  -> head-pair QKV projections (M=128)
  -> scores^T with the relative-position bias folded in as a rank-48
     factorization carried in constant contraction rows (even heads keep
     data in partitions 0:64 + consts 64:112; odd heads consts 16:64 + data
     64:128, so PSUM evictions never shift partitions)
  -> exp(s/8) per 125x500 PSUM bank (software-pipelined sc/AV streams)
  -> unnormalized attn @ [V | ones] (row-sums duplicated across 64
     partitions = free reciprocal broadcast) -> normalize -> Wm + ReLU with
  accum_out readout-sum -> standardize -> Wo.
"""

import sys

for p in ("/opt/trn_rl_repo", "/opt/pypackages"):
    if p not in sys.path:
        sys.path.insert(0, p)

import numpy as np

import concourse.bass as bass
import concourse.bacc as bacc
import concourse.tile as tile
import concourse.mybir as mybir
from concourse import bass_utils

F32 = mybir.dt.float32
F32R = mybir.dt.float32r
FP8 = mybir.dt.float8e4
DR = mybir.MatmulPerfMode.DoubleRow
AF = mybir.ActivationFunctionType
AX = mybir.AxisListType
ALU = mybir.AluOpType
ESHIFT = 2.0  # exp(x/8 - ESHIFT): range shift for fp8 attn weights

N_CORES = 8
B, L, C_IN = 64, 2000, 4
F, KW, PAD = 256, 13, 6
POOL = 4
H, DH = 8, 64
DM, NCLS = 100, 2
NOUT = L // POOL  # 500
BPC = B // N_CORES  # 8 samples per core
LP = L + 2 * PAD  # 2012
KC = F // 128  # 2 contraction chunks of 128
MSZ = 125  # NOUT split into 4 partition chunks of 125
RNK = 48  # rank of the bias factorization folded into the scores matmul
BN_N = float(B * L)  # batchnorm reduction count
# conv chunks of sample s+1 woven at each head of sample s
_WEAVE = {1: (0,), 2: (1,), 3: (2,), 4: (3,), 5: (4, 5), 6: (6, 7), 7: ()}


def _svd_bias_factors():
    """Rank-RNK factorization of -8*bias (added to raw qk before the 1/8
    exp scale). scoresT += ut.T @ wf; each returned block is [64, NOUT] with
    rows RNK:64 zero."""
    idx = np.arange(NOUT)
    target = -8.0 * (4.0 * np.abs(idx[:, None] - idx[None, :]) + 3.0) / (L - 1)
    U, S, Vt = np.linalg.svd(target)
    uf = (U[:, :RNK] * np.sqrt(S[:RNK])).astype(np.float32)  # [NOUT, RNK]
    wf = (np.sqrt(S[:RNK])[:, None] * Vt[:RNK]).astype(np.float32)
    ut = np.zeros((64, NOUT), np.float32)
    wz = np.zeros((64, NOUT), np.float32)
    ut[:RNK] = uf.T
    wz[:RNK] = wf
    return ut, wz


_SVD_UT, _SVD_WF = _svd_bias_factors()


def _build_program():
    nc = bacc.Bacc("TRN2", target_bir_lowering=False, debug=False,
                   num_devices=N_CORES)
    dram = {}

    def din(name, shape, dt=F32R):
        dram[name] = nc.dram_tensor(name, list(shape), dt,
                                    kind="ExternalInput").ap()
        return dram[name]

    din("imcm", [BPC, 4, C_IN * KW, NOUT])  # chunk-major contiguous im2col
    din("wc", [C_IN * KW, F])         # conv lhsT (52, 256), BN scale folded
    din("wq2", [128, KC * 4 * 128])   # [p, kc*512 + pr*128 + (head-in-pair d)]
    din("wk2", [128, KC * 4 * 128])
    din("wv", [128, KC * H * DH])     # [p, kc*512 + h*64 + d]
    din("wm", [128, 4 * DM])          # [p, c*100 + j]
    din("wo", [DM + 1, NCLS])         # [Wo; bo]
    din("svd_e", [64, NOUT])          # even-head const rows: [WF; zeros]
    din("svd_eu", [64, NOUT])         # even-head const rows: [UT; zeros]
    din("svd_o", [64, NOUT])          # odd-head const rows: [zeros; WF]
    din("svd_ou", [64, NOUT])         # odd-head const rows: [zeros; UT]
    din("bqk", [128, H], F32)         # pair-layout q/k biases
    din("bm_eff", [DM], F32)          # bm + Wm^T contribution of bv
    din("bia2", [128, 2], F32)        # host-exact BN bias (beta - mu*scale)
    out = nc.dram_tensor("out", [BPC, NCLS], F32, kind="ExternalOutput").ap()

    with tile.TileContext(nc) as tc:
        _emit(tc, dram, out)
    nc.compile()
    return nc


def _emit(tc, d, out):
    nc = tc.nc
    from contextlib import ExitStack

    ctx = ExitStack()
    with ctx:
        cst = ctx.enter_context(tc.tile_pool(name="cst", bufs=1))

        # ---- constants into SBUF ----
        bia_sb = cst.tile([128, 2], F32)
        nc.sync.dma_start(bia_sb[:], d["bia2"][:])  # tiny: warms the DMA ring
        wc_sb = cst.tile([C_IN * KW, F], F32R)
        nc.sync.dma_start(wc_sb[:], d["wc"][:])

        featn = cst.tile([DM, BPC], F32)
        ones_k = cst.tile([DM, 1], F32)
        nc.gpsimd.memset(ones_k[:], 1.0)
        ones_p = cst.tile([1, DM], F32)
        nc.gpsimd.memset(ones_p[:], 1.0)
        ones_fc = cst.tile([DM + 1, BPC], F32)
        nc.gpsimd.memset(ones_fc[:], 1.0)
        nsh = cst.tile([128, 1], F32)
        nc.gpsimd.memset(nsh[:], -ESHIFT)
        # persistent fp8 [V | ones] tiles for the DoubleRow attn@V matmuls:
        # 2 alternating sets of 2 pair-tiles [125, mcip*1024 + h*128 + d];
        # the ones regions are written once here, only V data is refreshed
        vts_sets = []
        for st in range(2):
            vset = []
            for pair in range(2):
                vt = cst.tile([MSZ, 2 * H * 128], FP8, name=f"vt{st}_{pair}")
                nc.gpsimd.memset(vt[:], 1.0)
                vset.append(vt)
            vts_sets.append(vset)

        with tc.tile_pool(name="imp", bufs=4) as imp, \
             tc.tile_pool(name="ymp", bufs=8) as ymp, \
             tc.tile_pool(name="htp", bufs=4) as htp, \
             tc.tile_pool(name="ptp", bufs=4) as ptp, \
             tc.tile_pool(name="rbp", bufs=3) as rbp, \
             tc.tile_pool(name="ocp", bufs=8) as ocp, \
             tc.tile_pool(name="msp", bufs=2) as msp, \
             tc.tile_pool(name="qkp", bufs=2, space="PSUM") as qkp, \
             tc.tile_pool(name="scp", bufs=2, space="PSUM") as scp, \
             tc.tile_pool(name="opp", bufs=2, space="PSUM") as opp:
            # ---- conv emission helpers (woven into the attention loop) ----
            imts, yms = [None] * BPC, [[None, None] for _ in range(BPC)]
            hts = [None] * BPC

            def conv_dma(s):
                # chunk-major contiguous im2col: one descriptor-light DMA per
                # 500-column chunk, each its own tile so the first conv matmul
                # only waits on its own chunk's DMA
                tiles = []
                for c in range(4):
                    t = imp.tile([C_IN * KW, NOUT], F32R, tag="imt",
                                 name="imt")
                    nc.sync.dma_start(t[:], d["imcm"][s][c])
                    tiles.append(t)
                imts[s] = tiles

            def conv_chunk(s, idx, pool=None):
                fh, c = idx // 4, idx % 4
                if yms[s][fh] is None:
                    yms[s][fh] = ymp.tile([128, NOUT], F32, tag="ym", name="ym")
                pl, tg = pool if pool is not None else (qkp, "pqk")
                ps = pl.tile([128, NOUT], F32, tag=tg, name="cvp")
                nc.tensor.matmul(
                    ps[:], wc_sb[:, fh * 128:(fh + 1) * 128],
                    imts[s][c][:], start=True, stop=True)
                nc.vector.reduce_max(
                    yms[s][fh][:, c * 125:(c + 1) * 125],
                    ps[:].rearrange("p (a b) -> p a b", b=POOL), axis=AX.X)

            def bn_relu(s):
                # pooled conv -> fused bias + ReLU on ACT (BN scale is folded
                # into the conv weights host-side)
                ht = []
                for fh in range(2):
                    t = htp.tile([128, NOUT], F32R, tag="ht", name="ht")
                    nc.scalar.activation(t[:], yms[s][fh][:], AF.Relu,
                                         bias=bia_sb[:, fh:fh + 1])
                    ht.append(t)
                hts[s] = ht

            # weight/const DMAs spread across queues so the sample-0 im2col
            # stream on the sync queue isn't head-of-line blocked
            wv_sb = cst.tile([128, KC * H * DH], F32R)
            nc.scalar.dma_start(wv_sb[:], d["wv"][:])
            wq_sb = cst.tile([128, KC * 4 * 128], F32R)
            nc.scalar.dma_start(wq_sb[:], d["wq2"][:])
            wk_sb = cst.tile([128, KC * 4 * 128], F32R)
            nc.scalar.dma_start(wk_sb[:], d["wk2"][:])
            wm_sb = cst.tile([128, 4 * DM], F32R)
            nc.scalar.dma_start(wm_sb[:], d["wm"][:])
            wo_sb = cst.tile([DM + 1, NCLS], F32R)
            nc.scalar.dma_start(wo_sb[:], d["wo"][:])
            bm_sb = cst.tile([DM, 1], F32)
            nc.scalar.dma_start(bm_sb[:], d["bm_eff"][:])

            # prologue: sample 0 conv (column-major so each im2col chunk DMA
            # feeds two matmuls as soon as it lands)
            conv_dma(0)
            bqk_sb = cst.tile([128, H], F32)
            nc.sync.dma_start(bqk_sb[:], d["bqk"][:])
            qte, qto, kte, kto = [], [], [], []
            for i in range(2):
                te = cst.tile([128, NOUT], F32R, name=f"qte{i}")
                nc.sync.dma_start(te[64:128, :], d["svd_e"][:])
                qte.append(te)
                to = cst.tile([128, NOUT], F32R, name=f"qto{i}")
                nc.sync.dma_start(to[0:64, :], d["svd_o"][:])
                qto.append(to)
                ke = cst.tile([128, NOUT], F32R, name=f"kte{i}")
                nc.sync.dma_start(ke[64:128, :], d["svd_eu"][:])
                kte.append(ke)
                ko = cst.tile([128, NOUT], F32R, name=f"kto{i}")
                nc.sync.dma_start(ko[0:64, :], d["svd_ou"][:])
                kto.append(ko)
            _pro = [(qkp, "pqk"), (opp, "op")]
            for i, idx in enumerate((0, 4, 1, 5, 2, 6, 3, 7)):
                conv_chunk(0, idx, _pro[i % 2])
            bn_relu(0)
            conv_dma(1)

            # ---- main loop: attention, software-pipelined ACROSS samples ----
            ocss = [None] * BPC

            # scores+exp for head h are emitted a head ahead of its attn@V,
            # so the exp latency hides under the next head's score matmuls;
            # the last head's attn@V drains inside the NEXT sample
            def emit_sc(s, h):
                bb = (h // 2) % 2
                qt_t = qte[bb] if h % 2 == 0 else qto[bb]
                kt_t = kte[bb] if h % 2 == 0 else kto[bb]
                op = opp.tile([128, NOUT], F32, tag="op", name="op",
                              padded_shape=[128, 512])
                pts = []
                for half in range(2):
                    sct = scp.tile([MSZ, 1024], F32, tag="sc",
                                   name="sct", padded_shape=[128, 1024])
                    for j in range(2):
                        m0 = (half * 2 + j) * MSZ
                        nc.tensor.matmul(
                            sct[:, j * 512:j * 512 + NOUT],
                            kt_t[:, m0:m0 + MSZ], qt_t[:],
                            start=True, stop=True)
                    pt = ptp.tile([MSZ, 1024], FP8, tag="pt", name="pt")
                    nc.scalar.activation(
                        pt[:].rearrange("p (b x) -> p b x", b=2)[:, :, 0:NOUT],
                        sct[:].rearrange("p (b x) -> p b x", b=2)[:, :, 0:NOUT],
                        AF.Exp, scale=1.0 / 8.0, bias=nsh[0:MSZ, :])
                    pts.append(pt)
                return (s, h, op, pts)

            def emit_av(st):
                s_, h, op, pts = st
                vv = vts_sets[s_ % 2]
                for half in range(2):
                    nc.tensor.matmul(
                        op[:, 0:NOUT],
                        vv[half][:].rearrange(
                            "p (b x) -> p b x", b=2)[:, :, h * 128:(h + 1) * 128],
                        pts[half][:].rearrange(
                            "p (b x) -> p b x", b=2)[:, :, 0:NOUT],
                        start=(half == 0), stop=(half == 1),
                        perf_mode=DR)
                rb = rbp.tile([128, NOUT], F32, tag="rb", name="rb")
                nc.vector.reciprocal_approx_fast(rb[:], op[:, 0:NOUT])
                nc.vector.tensor_mul(
                    ocss[s_][h // 2][(h % 2) * 64:(h % 2) * 64 + 64, :],
                    op[0:64, 0:NOUT], rb[64:128, :])

            def finish_sample(sp):
                mp = scp.tile([DM, NOUT], F32, tag="sc", name="mp",
                              padded_shape=[128, 512])
                for c in range(4):
                    nc.tensor.matmul(mp[:], wm_sb[:, c * DM:(c + 1) * DM],
                                     ocss[sp][c][:], start=(c == 0),
                                     stop=(c == 3))
                ms = msp.tile([DM, NOUT], F32)
                nc.scalar.activation(ms[:], mp[:], AF.Relu, bias=bm_sb[:],
                                     accum_out=featn[:, sp:sp + 1])

            pend = None
            for s in range(BPC):
                if s + 2 < BPC:
                    conv_dma(s + 2)
                ht = hts[s]
                vts = vts_sets[s % 2]
                ocss[s] = [ocp.tile([128, NOUT], F32R, tag="oc", name="oc")
                           for _ in range(4)]

                # V for all heads -> fp8 pair tiles [m, (mcip, 64 v|64 ones)];
                # vp lives in the qkp banks (free until this sample's pq)
                for mc in range(4):
                    vp = qkp.tile([MSZ, 512], F32, tag="pqk", name="vp",
                                  padded_shape=[128, 512])
                    m0 = mc * MSZ
                    for kc in range(KC):
                        nc.tensor.matmul(
                            vp[:, 0:H * DH], ht[kc][:, m0:m0 + MSZ],
                            wv_sb[:, kc * 512:(kc + 1) * 512],
                            start=(kc == 0), stop=(kc == KC - 1))
                    dst = vts[mc // 2][:, (mc % 2) * 1024:(mc % 2) * 1024 + 1024]
                    nc.vector.tensor_copy(
                        dst.rearrange("p (h x) -> p h x", x=128)[:, :, 0:DH],
                        vp[:, 0:H * DH].rearrange("p (h x) -> p h x", x=DH))

                for pr in range(4):
                    bb = pr % 2
                    pq = qkp.tile([128, NOUT], F32, tag="pqk", name="pq")
                    for kc in range(KC):
                        nc.tensor.matmul(
                            pq[:], wq_sb[:, kc * 512 + pr * 128:
                                         kc * 512 + (pr + 1) * 128],
                            ht[kc][:], start=(kc == 0), stop=(kc == KC - 1))
                    # evictions: even head -> rows 0:64, odd -> rows 64:128
                    nc.scalar.activation(
                        qte[bb][0:64, :], pq[0:64, :], AF.Identity,
                        bias=bqk_sb[0:64, 2 * pr:2 * pr + 1])
                    nc.vector.tensor_scalar_add(
                        qto[bb][64:128, :], pq[64:128, :],
                        bqk_sb[64:128, 2 * pr:2 * pr + 1])
                    pk = qkp.tile([128, NOUT], F32, tag="pqk", name="pk")
                    for kc in range(KC):
                        nc.tensor.matmul(
                            pk[:], wk_sb[:, kc * 512 + pr * 128:
                                         kc * 512 + (pr + 1) * 128],
                            ht[kc][:], start=(kc == 0), stop=(kc == KC - 1))
                    nc.vector.tensor_scalar_add(
                        kte[bb][0:64, :], pk[0:64, :],
                        bqk_sb[0:64, 2 * pr + 1:2 * pr + 2])
                    nc.scalar.activation(
                        kto[bb][0 + 64:128, :], pk[64:128, :], AF.Identity,
                        bias=bqk_sb[64:128, 2 * pr + 1:2 * pr + 2])

                    for h in (2 * pr, 2 * pr + 1):
                        st = emit_sc(s, h)
                        if pend is not None:
                            emit_av(pend)
                            if h == 0:
                                finish_sample(s - 1)
                            # weave next sample's conv so its pool + bias+ReLU
                            # clear the DVE/ACT queues well before the boundary
                            if s + 1 < BPC:
                                for cidx in _WEAVE.get(h, ()):
                                    conv_chunk(s + 1, cidx)
                                if h == 6:
                                    bn_relu(s + 1)
                        pend = st
            emit_av(pend)
            finish_sample(BPC - 1)

        # ---- final: standardize + Wo ----
        with tc.tile_pool(name="fsp", bufs=1) as fsp, \
             tc.tile_pool(name="fpp", bufs=1, space="PSUM") as fpp:
            fsq = fsp.tile([DM, BPC], F32)
            nc.vector.tensor_mul(fsq[:], featn[:], featn[:])
            cs = fpp.tile([1, 2 * BPC], F32, tag="cs")
            nc.tensor.matmul(cs[:, 0:BPC], ones_k[:], featn[:], start=True,
                             stop=True)
            nc.tensor.matmul(cs[:, BPC:2 * BPC], ones_k[:], fsq[:], start=True,
                             stop=True)
            st = fsp.tile([1, 4 * BPC], F32)
            nc.vector.tensor_scalar_mul(st[:, 0:BPC], cs[:, 0:BPC], 1.0 / DM)
            nc.vector.tensor_scalar_mul(st[:, BPC:2 * BPC], cs[:, BPC:2 * BPC],
                                        1.0 / DM)
            nc.vector.tensor_mul(st[:, 2 * BPC:3 * BPC], st[:, 0:BPC],
                                 st[:, 0:BPC])
            nc.vector.tensor_sub(st[:, 3 * BPC:4 * BPC], st[:, BPC:2 * BPC],
                                 st[:, 2 * BPC:3 * BPC])
            # 1/(sqrt(v)+1e-6) ~= exp(-0.5*ln(v)) (v ~ O(1e4+), eps negligible;
            # ln+exp share the already-loaded natural_log_exp table set)
            sdt = fsp.tile([1, BPC], F32, tag="sdt")
            nc.scalar.activation(sdt[:], st[:, 3 * BPC:4 * BPC], AF.Ln)
            rsd = fsp.tile([1, BPC], F32, tag="rsd")
            nc.scalar.activation(rsd[:], sdt[:], AF.Exp, scale=-0.5)
            bcm = fpp.tile([DM, BPC], F32, tag="bcm")
            nc.tensor.matmul(bcm[:], ones_p[:], st[:, 0:BPC], start=True,
                             stop=True)
            bcr = fpp.tile([DM, BPC], F32, tag="bcr")
            nc.tensor.matmul(bcr[:], ones_p[:], rsd[:], start=True, stop=True)
            fc = fsp.tile([DM, BPC], F32, tag="fc")
            nc.vector.tensor_sub(fc[:], featn[:], bcm[:])
            fcn = fsp.tile([DM + 1, BPC], F32R, tag="fcn")
            nc.vector.tensor_copy(fcn[:], ones_fc[:])
            nc.vector.tensor_mul(fcn[0:DM, :], fc[:], bcr[:])
            fo = fpp.tile([BPC, NCLS], F32, tag="fo")
            nc.tensor.matmul(fo[:], fcn[:], wo_sb[:], start=True, stop=True)
            osb = fsp.tile([BPC, NCLS], F32, tag="osb")
            nc.vector.tensor_copy(osb[:], fo[:])
            nc.sync.dma_start(out[:], osb[:])


_NC_CACHE = None


def _get_program():
    global _NC_CACHE
    if _NC_CACHE is None:
        _NC_CACHE = _build_program()
    return _NC_CACHE


def _prep_inputs(x, conv_w, bn_gamma, bn_beta, Wq, bq, Wk, bk, Wv, bv, Wm, bm,
                 Wo, bo):
    f32 = np.float32
    x = np.asarray(x, f32)
    xpad = np.zeros((B, C_IN, LP), f32)
    xpad[:, :, PAD:PAD + L] = np.transpose(x, (0, 2, 1))
    wc = np.ascontiguousarray(
        np.transpose(np.asarray(conv_w, f32), (1, 2, 0)).reshape(C_IN * KW, F))

    # exact batch statistics of the conv output, on the host (fp64): the
    # column-sum and Gram matrix of the full-batch im2col reduce the
    # [B*L]-point mean/var to 52-dim quadratic forms.
    win = np.lib.stride_tricks.sliding_window_view(xpad, KW, axis=2)  # B,C,L,K
    im_all = np.ascontiguousarray(
        win.transpose(0, 1, 3, 2).reshape(B, C_IN * KW, L))
    # chunk-major contiguous layout: each [52, 500] chunk is one cheap DMA
    imcm = np.ascontiguousarray(
        im_all.reshape(B, C_IN * KW, 4, NOUT).transpose(0, 2, 1, 3))
    a = im_all.transpose(1, 0, 2).reshape(C_IN * KW, B * L).astype(np.float64)
    imsum = a.sum(axis=1)  # [52]
    gram = a @ a.T         # [52, 52]
    wc64 = wc.astype(np.float64)
    mu = (imsum @ wc64) / BN_N                            # [256]
    var = np.einsum("kf,kj,jf->f", wc64, gram, wc64) / BN_N - mu * mu
    scl = np.asarray(bn_gamma, np.float64) / np.sqrt(var + 1e-5)
    bia = np.asarray(bn_beta, np.float64) - mu * scl
    # BN scale folds into the conv weights (scale > 0 commutes with the
    # maxpool that runs before the bias+ReLU on device)
    wc_scaled = (wc64 * scl[None, :]).astype(f32)
    bia2 = np.ascontiguousarray(bia.astype(f32).reshape(2, 128).T)

    def pair_layout(W):  # [H, F, DH] -> [128, kc*512 + pr*128 + (64h0|64h1)]
        W = np.asarray(W, f32).reshape(4, 2, KC, 128, DH)  # pr, hp, kc, p, d
        o = np.transpose(W, (3, 2, 0, 1, 4))  # p, kc, pr, hp, d
        return np.ascontiguousarray(o.reshape(128, KC * 4 * 128))

    wq2, wk2 = pair_layout(Wq), pair_layout(Wk)
    wvh = np.ascontiguousarray(
        np.transpose(np.asarray(Wv, f32).reshape(H, KC, 128, DH),
                     (2, 1, 0, 3)).reshape(128, KC * H * DH))
    wmh = np.ascontiguousarray(
        np.transpose(np.asarray(Wm, f32).reshape(4, 128, DM), (1, 0, 2))
        .reshape(128, 4 * DM))
    woh = np.concatenate([np.asarray(Wo, f32),
                          np.asarray(bo, f32)[None, :]], axis=0)
    # pair-layout biases: col 2pr = [bq[2pr]; bq[2pr+1]], col 2pr+1 for bk
    bq_, bk_ = np.asarray(bq, f32), np.asarray(bk, f32)
    bqkh = np.zeros((128, H), f32)
    for pr in range(4):
        bqkh[0:64, 2 * pr] = bq_[2 * pr]
        bqkh[64:128, 2 * pr] = bq_[2 * pr + 1]
        bqkh[0:64, 2 * pr + 1] = bk_[2 * pr]
        bqkh[64:128, 2 * pr + 1] = bk_[2 * pr + 1]
    bv_f = np.asarray(bv, f32).reshape(H * DH)
    bmh = np.asarray(bm, f32) + bv_f @ np.asarray(Wm, f32)

    # odd-head const layouts: [16 zero pad; 48 factor rows]
    svd_o = np.zeros((64, NOUT), f32)
    svd_o[16:64] = _SVD_WF[0:RNK]
    svd_ou = np.zeros((64, NOUT), f32)
    svd_ou[16:64] = _SVD_UT[0:RNK]

    shared = dict(wc=wc_scaled, wq2=wq2, wk2=wk2, wv=wvh, wm=wmh, wo=woh,
                  bqk=bqkh, bm_eff=bmh, bia2=bia2,
                  svd_e=_SVD_WF, svd_eu=_SVD_UT, svd_o=svd_o, svd_ou=svd_ou)
    in_maps = []
    for c in range(N_CORES):
        m = dict(shared)
        m["imcm"] = np.ascontiguousarray(imcm[c * BPC:(c + 1) * BPC])
        in_maps.append(m)
    return in_maps


def kernel(**inputs):
    in_maps = _prep_inputs(**inputs)
    nc = _get_program()
    res = bass_utils.run_bass_kernel_spmd(nc, in_maps, list(range(N_CORES)))
    return np.concatenate([res.results[i]["out"] for i in range(N_CORES)],
                          axis=0).astype(np.float32)


# revision 35
# speedup vs baseline: 408.0479x; 1.0065x over previous
"""Trainium2 Bass kernel for AttentionNet (conv frontend + MHA + readout).

Strategy: pure data-parallel over batch (64 samples -> 8 cores x 8). BatchNorm
batch statistics are computed exactly on the host from the im2col column-sum /
Gram aggregates (the affine scale folds into the conv weights, which commutes
with maxpool+ReLU because scale > 0), so there is no on-device collective and
no BN-stats prologue at all. All heavy matmuls in fp32r.

Per-core pipeline:
  conv1d (im2col matmul, K=52) -> maxpool(4) -> fused bias+ReLU (ACT)
  -> head-pair QKV projections (M=128)
  -> scores^T with the relative-position bias folded in as a rank-48
     factorization carried in constant contraction rows (even heads keep
     data in partitions 0:64 + consts 64:112; odd heads consts 16:64 + data
     64:128, so PSUM evictions never shift partitions)
  -> exp(s/8) per 125x500 PSUM bank (software-pipelined sc/AV streams)
  -> unnormalized attn @ [V | ones] (row-sums duplicated across 64
     partitions = free reciprocal broadcast) -> normalize -> Wm + ReLU with
  accum_out readout-sum -> standardize -> Wo.
"""

import sys

for p in ("/opt/trn_rl_repo", "/opt/pypackages"):
    if p not in sys.path:
        sys.path.insert(0, p)

import numpy as np

import concourse.bass as bass
import concourse.bacc as bacc
import concourse.tile as tile
import concourse.mybir as mybir
from concourse import bass_utils

F32 = mybir.dt.float32
F32R = mybir.dt.float32r
FP8 = mybir.dt.float8e4
DR = mybir.MatmulPerfMode.DoubleRow
AF = mybir.ActivationFunctionType
AX = mybir.AxisListType
ALU = mybir.AluOpType
ESHIFT = 1.0  # exp(x/8 - ESHIFT): range shift for fp8 attn weights
VSCALE = 16.0  # V pre-scale lifting small values out of fp8 subnormals

N_CORES = 8
B, L, C_IN = 64, 2000, 4
F, KW, PAD = 256, 13, 6
POOL = 4
H, DH = 8, 64
DM, NCLS = 100, 2
NOUT = L // POOL  # 500
BPC = B // N_CORES  # 8 samples per core
LP = L + 2 * PAD  # 2012
KC = F // 128  # 2 contraction chunks of 128
MSZ = 125  # NOUT split into 4 partition chunks of 125
RNK = 48  # rank of the bias factorization folded into the scores matmul
BN_N = float(B * L)  # batchnorm reduction count
# conv chunks of sample s+1 woven at each head of sample s
_WEAVE = {1: (0,), 2: (1,), 3: (2,), 4: (3,), 5: (4, 5), 6: (6, 7), 7: ()}


def _svd_bias_factors():
    """Rank-RNK factorization of -8*bias (added to raw qk before the 1/8
    exp scale). scoresT += ut.T @ wf; each returned block is [64, NOUT] with
    rows RNK:64 zero."""
    idx = np.arange(NOUT)
    target = -8.0 * (4.0 * np.abs(idx[:, None] - idx[None, :]) + 3.0) / (L - 1)
    U, S, Vt = np.linalg.svd(target)
    uf = (U[:, :RNK] * np.sqrt(S[:RNK])).astype(np.float32)  # [NOUT, RNK]
    wf = (np.sqrt(S[:RNK])[:, None] * Vt[:RNK]).astype(np.float32)
    ut = np.zeros((64, NOUT), np.float32)
    wz = np.zeros((64, NOUT), np.float32)
    ut[:RNK] = uf.T
    wz[:RNK] = wf
    return ut, wz


_SVD_UT, _SVD_WF = _svd_bias_factors()


def _build_program():
    nc = bacc.Bacc("TRN2", target_bir_lowering=False, debug=False,
                   num_devices=N_CORES)
    dram = {}

    def din(name, shape, dt=F32R):
        dram[name] = nc.dram_tensor(name, list(shape), dt,
                                    kind="ExternalInput").ap()
        return dram[name]

    din("imcm", [BPC, 4, C_IN * KW, NOUT])  # chunk-major contiguous im2col
    din("wc", [C_IN * KW, F])         # conv lhsT (52, 256), BN scale folded
    din("wq2", [128, KC * 4 * 128])   # [p, kc*512 + pr*128 + (head-in-pair d)]
    din("wk2", [128, KC * 4 * 128])
    din("wv", [128, KC * H * DH])     # [p, kc*512 + h*64 + d]
    din("wm", [128, 4 * DM])          # [p, c*100 + j]
    din("wo", [DM + 1, NCLS])         # [Wo; bo]
    din("svd_e", [64, NOUT])          # even-head const rows: [WF; zeros]
    din("svd_eu", [64, NOUT])         # even-head const rows: [UT; zeros]
    din("svd_o", [64, NOUT])          # odd-head const rows: [zeros; WF]
    din("svd_ou", [64, NOUT])         # odd-head const rows: [zeros; UT]
    din("bqk", [128, H], F32)         # pair-layout q/k biases
    din("bm_eff", [DM], F32)          # bm + Wm^T contribution of bv
    din("bia2", [128, 2], F32)        # host-exact BN bias (beta - mu*scale)
    out = nc.dram_tensor("out", [BPC, NCLS], F32, kind="ExternalOutput").ap()

    with tile.TileContext(nc) as tc:
        _emit(tc, dram, out)
    nc.compile()
    return nc


def _emit(tc, d, out):
    nc = tc.nc
    from contextlib import ExitStack

    ctx = ExitStack()
    with ctx:
        cst = ctx.enter_context(tc.tile_pool(name="cst", bufs=1))

        # ---- constants into SBUF ----
        bia_sb = cst.tile([128, 2], F32)
        nc.sync.dma_start(bia_sb[:], d["bia2"][:])  # tiny: warms the DMA ring
        wc_sb = cst.tile([C_IN * KW, F], F32R)
        nc.sync.dma_start(wc_sb[:], d["wc"][:])

        featn = cst.tile([DM, BPC], F32)
        ones_k = cst.tile([DM, 1], F32)
        nc.gpsimd.memset(ones_k[:], 1.0)
        ones_p = cst.tile([1, DM], F32)
        nc.gpsimd.memset(ones_p[:], 1.0)
        ones_fc = cst.tile([DM + 1, BPC], F32)
        nc.gpsimd.memset(ones_fc[:], 1.0)
        nsh = cst.tile([128, 1], F32)
        nc.gpsimd.memset(nsh[:], -ESHIFT)
        # persistent fp8 [V | ones] tiles for the DoubleRow attn@V matmuls:
        # 2 alternating sets of 2 pair-tiles [125, mcip*1024 + h*128 + d];
        # the ones regions are written once here, only V data is refreshed
        vts_sets = []
        for st in range(2):
            vset = []
            for pair in range(2):
                vt = cst.tile([MSZ, 2 * H * 128], FP8, name=f"vt{st}_{pair}")
                nc.gpsimd.memset(vt[:], 1.0)
                vset.append(vt)
            vts_sets.append(vset)

        with tc.tile_pool(name="imp", bufs=4) as imp, \
             tc.tile_pool(name="ymp", bufs=8) as ymp, \
             tc.tile_pool(name="htp", bufs=4) as htp, \
             tc.tile_pool(name="ptp", bufs=4) as ptp, \
             tc.tile_pool(name="rbp", bufs=3) as rbp, \
             tc.tile_pool(name="ocp", bufs=8) as ocp, \
             tc.tile_pool(name="msp", bufs=2) as msp, \
             tc.tile_pool(name="qkp", bufs=2, space="PSUM") as qkp, \
             tc.tile_pool(name="scp", bufs=2, space="PSUM") as scp, \
             tc.tile_pool(name="opp", bufs=2, space="PSUM") as opp:
            # ---- conv emission helpers (woven into the attention loop) ----
            imts, yms = [None] * BPC, [[None, None] for _ in range(BPC)]
            hts = [None] * BPC

            def conv_dma(s):
                # chunk-major contiguous im2col: one descriptor-light DMA per
                # 500-column chunk, each its own tile so the first conv matmul
                # only waits on its own chunk's DMA
                tiles = []
                for c in range(4):
                    t = imp.tile([C_IN * KW, NOUT], F32R, tag="imt",
                                 name="imt")
                    nc.sync.dma_start(t[:], d["imcm"][s][c])
                    tiles.append(t)
                imts[s] = tiles

            def conv_chunk(s, idx, pool=None):
                fh, c = idx // 4, idx % 4
                if yms[s][fh] is None:
                    yms[s][fh] = ymp.tile([128, NOUT], F32, tag="ym", name="ym")
                pl, tg = pool if pool is not None else (qkp, "pqk")
                ps = pl.tile([128, NOUT], F32, tag=tg, name="cvp")
                nc.tensor.matmul(
                    ps[:], wc_sb[:, fh * 128:(fh + 1) * 128],
                    imts[s][c][:], start=True, stop=True)
                nc.vector.reduce_max(
                    yms[s][fh][:, c * 125:(c + 1) * 125],
                    ps[:].rearrange("p (a b) -> p a b", b=POOL), axis=AX.X)

            def bn_relu(s):
                # pooled conv -> fused bias + ReLU on ACT (BN scale is folded
                # into the conv weights host-side)
                ht = []
                for fh in range(2):
                    t = htp.tile([128, NOUT], F32R, tag="ht", name="ht")
                    nc.scalar.activation(t[:], yms[s][fh][:], AF.Relu,
                                         bias=bia_sb[:, fh:fh + 1])
                    ht.append(t)
                hts[s] = ht

            # weight/const DMAs spread across queues so the sample-0 im2col
            # stream on the sync queue isn't head-of-line blocked
            wv_sb = cst.tile([128, KC * H * DH], F32R)
            nc.scalar.dma_start(wv_sb[:], d["wv"][:])
            wq_sb = cst.tile([128, KC * 4 * 128], F32R)
            nc.scalar.dma_start(wq_sb[:], d["wq2"][:])
            wk_sb = cst.tile([128, KC * 4 * 128], F32R)
            nc.scalar.dma_start(wk_sb[:], d["wk2"][:])
            wm_sb = cst.tile([128, 4 * DM], F32R)
            nc.scalar.dma_start(wm_sb[:], d["wm"][:])
            wo_sb = cst.tile([DM + 1, NCLS], F32R)
            nc.scalar.dma_start(wo_sb[:], d["wo"][:])
            bm_sb = cst.tile([DM, 1], F32)
            nc.scalar.dma_start(bm_sb[:], d["bm_eff"][:])

            # prologue: sample 0 conv (column-major so each im2col chunk DMA
            # feeds two matmuls as soon as it lands)
            conv_dma(0)
            bqk_sb = cst.tile([128, H], F32)
            nc.sync.dma_start(bqk_sb[:], d["bqk"][:])
            qte, qto, kte, kto = [], [], [], []
            for i in range(2):
                te = cst.tile([128, NOUT], F32R, name=f"qte{i}")
                nc.sync.dma_start(te[64:128, :], d["svd_e"][:])
                qte.append(te)
                to = cst.tile([128, NOUT], F32R, name=f"qto{i}")
                nc.sync.dma_start(to[0:64, :], d["svd_o"][:])
                qto.append(to)
                ke = cst.tile([128, NOUT], F32R, name=f"kte{i}")
                nc.sync.dma_start(ke[64:128, :], d["svd_eu"][:])
                kte.append(ke)
                ko = cst.tile([128, NOUT], F32R, name=f"kto{i}")
                nc.sync.dma_start(ko[0:64, :], d["svd_ou"][:])
                kto.append(ko)
            _pro = [(qkp, "pqk"), (opp, "op")]
            for i, idx in enumerate((0, 4, 1, 5, 2, 6, 3, 7)):
                conv_chunk(0, idx, _pro[i % 2])
            bn_relu(0)
            conv_dma(1)

            # ---- main loop: attention, software-pipelined ACROSS samples ----
            ocss = [None] * BPC

            # scores+exp for head h are emitted a head ahead of its attn@V,
            # so the exp latency hides under the next head's score matmuls;
            # the last head's attn@V drains inside the NEXT sample
            def emit_sc(s, h):
                bb = (h // 2) % 2
                qt_t = qte[bb] if h % 2 == 0 else qto[bb]
                kt_t = kte[bb] if h % 2 == 0 else kto[bb]
                op = opp.tile([128, NOUT], F32, tag="op", name="op",
                              padded_shape=[128, 512])
                pts = []
                for half in range(2):
                    sct = scp.tile([MSZ, 1024], F32, tag="sc",
                                   name="sct", padded_shape=[128, 1024])
                    for j in range(2):
                        m0 = (half * 2 + j) * MSZ
                        nc.tensor.matmul(
                            sct[:, j * 512:j * 512 + NOUT],
                            kt_t[:, m0:m0 + MSZ], qt_t[:],
                            start=True, stop=True)
                    pt = ptp.tile([MSZ, 1024], FP8, tag="pt", name="pt")
                    nc.scalar.activation(
                        pt[:].rearrange("p (b x) -> p b x", b=2)[:, :, 0:NOUT],
                        sct[:].rearrange("p (b x) -> p b x", b=2)[:, :, 0:NOUT],
                        AF.Exp, scale=1.0 / 8.0, bias=nsh[0:MSZ, :])
                    pts.append(pt)
                return (s, h, op, pts)

            def emit_av(st):
                s_, h, op, pts = st
                vv = vts_sets[s_ % 2]
                for half in range(2):
                    nc.tensor.matmul(
                        op[:, 0:NOUT],
                        vv[half][:].rearrange(
                            "p (b x) -> p b x", b=2)[:, :, h * 128:(h + 1) * 128],
                        pts[half][:].rearrange(
                            "p (b x) -> p b x", b=2)[:, :, 0:NOUT],
                        start=(half == 0), stop=(half == 1),
                        perf_mode=DR)
                rb = rbp.tile([128, NOUT], F32, tag="rb", name="rb")
                nc.vector.reciprocal_approx_fast(rb[:], op[:, 0:NOUT])
                nc.vector.scalar_tensor_tensor(
                    ocss[s_][h // 2][(h % 2) * 64:(h % 2) * 64 + 64, :],
                    op[0:64, 0:NOUT], 1.0 / VSCALE, rb[64:128, :],
                    op0=ALU.mult, op1=ALU.mult)

            def finish_sample(sp):
                mp = scp.tile([DM, NOUT], F32, tag="sc", name="mp",
                              padded_shape=[128, 512])
                for c in range(4):
                    nc.tensor.matmul(mp[:], wm_sb[:, c * DM:(c + 1) * DM],
                                     ocss[sp][c][:], start=(c == 0),
                                     stop=(c == 3))
                ms = msp.tile([DM, NOUT], F32)
                nc.scalar.activation(ms[:], mp[:], AF.Relu, bias=bm_sb[:],
                                     accum_out=featn[:, sp:sp + 1])

            pend = None
            for s in range(BPC):
                if s + 2 < BPC:
                    conv_dma(s + 2)
                ht = hts[s]
                vts = vts_sets[s % 2]
                ocss[s] = [ocp.tile([128, NOUT], F32R, tag="oc", name="oc")
                           for _ in range(4)]

                # V for all heads -> fp8 pair tiles [m, (mcip, 64 v|64 ones)];
                # vp lives in the qkp banks (free until this sample's pq)
                for mc in range(4):
                    vp = qkp.tile([MSZ, 512], F32, tag="pqk", name="vp",
                                  padded_shape=[128, 512])
                    m0 = mc * MSZ
                    for kc in range(KC):
                        nc.tensor.matmul(
                            vp[:, 0:H * DH], ht[kc][:, m0:m0 + MSZ],
                            wv_sb[:, kc * 512:(kc + 1) * 512],
                            start=(kc == 0), stop=(kc == KC - 1))
                    dst = vts[mc // 2][:, (mc % 2) * 1024:(mc % 2) * 1024 + 1024]
                    nc.vector.tensor_copy(
                        dst.rearrange("p (h x) -> p h x", x=128)[:, :, 0:DH],
                        vp[:, 0:H * DH].rearrange("p (h x) -> p h x", x=DH))

                for pr in range(4):
                    bb = pr % 2
                    pq = qkp.tile([128, NOUT], F32, tag="pqk", name="pq")
                    for kc in range(KC):
                        nc.tensor.matmul(
                            pq[:], wq_sb[:, kc * 512 + pr * 128:
                                         kc * 512 + (pr + 1) * 128],
                            ht[kc][:], start=(kc == 0), stop=(kc == KC - 1))
                    # evictions: even head -> rows 0:64, odd -> rows 64:128
                    nc.scalar.activation(
                        qte[bb][0:64, :], pq[0:64, :], AF.Identity,
                        bias=bqk_sb[0:64, 2 * pr:2 * pr + 1])
                    nc.vector.tensor_scalar_add(
                        qto[bb][64:128, :], pq[64:128, :],
                        bqk_sb[64:128, 2 * pr:2 * pr + 1])
                    pk = qkp.tile([128, NOUT], F32, tag="pqk", name="pk")
                    for kc in range(KC):
                        nc.tensor.matmul(
                            pk[:], wk_sb[:, kc * 512 + pr * 128:
                                         kc * 512 + (pr + 1) * 128],
                            ht[kc][:], start=(kc == 0), stop=(kc == KC - 1))
                    nc.vector.tensor_scalar_add(
                        kte[bb][0:64, :], pk[0:64, :],
                        bqk_sb[0:64, 2 * pr + 1:2 * pr + 2])
                    nc.scalar.activation(
                        kto[bb][0 + 64:128, :], pk[64:128, :], AF.Identity,
                        bias=bqk_sb[64:128, 2 * pr + 1:2 * pr + 2])

                    for h in (2 * pr, 2 * pr + 1):
                        st = emit_sc(s, h)
                        if pend is not None:
                            emit_av(pend)
                            if h == 0:
                                finish_sample(s - 1)
                            # weave next sample's conv so its pool + bias+ReLU
                            # clear the DVE/ACT queues well before the boundary
                            if s + 1 < BPC:
                                for cidx in _WEAVE.get(h, ()):
                                    conv_chunk(s + 1, cidx)
                                if h == 6:
                                    bn_relu(s + 1)
                        pend = st
            emit_av(pend)
            finish_sample(BPC - 1)

        # ---- final: standardize + Wo ----
        with tc.tile_pool(name="fsp", bufs=1) as fsp, \
             tc.tile_pool(name="fpp", bufs=1, space="PSUM") as fpp:
            fsq = fsp.tile([DM, BPC], F32)
            nc.vector.tensor_mul(fsq[:], featn[:], featn[:])
            cs = fpp.tile([1, 2 * BPC], F32, tag="cs")
            nc.tensor.matmul(cs[:, 0:BPC], ones_k[:], featn[:], start=True,
                             stop=True)
            nc.tensor.matmul(cs[:, BPC:2 * BPC], ones_k[:], fsq[:], start=True,
                             stop=True)
            st = fsp.tile([1, 4 * BPC], F32)
            nc.vector.tensor_scalar_mul(st[:, 0:BPC], cs[:, 0:BPC], 1.0 / DM)
            nc.vector.tensor_scalar_mul(st[:, BPC:2 * BPC], cs[:, BPC:2 * BPC],
                                        1.0 / DM)
            nc.vector.tensor_mul(st[:, 2 * BPC:3 * BPC], st[:, 0:BPC],
                                 st[:, 0:BPC])
            nc.vector.tensor_sub(st[:, 3 * BPC:4 * BPC], st[:, BPC:2 * BPC],
                                 st[:, 2 * BPC:3 * BPC])
            # 1/(sqrt(v)+1e-6) ~= exp(-0.5*ln(v)) (v ~ O(1e4+), eps negligible;
            # ln+exp share the already-loaded natural_log_exp table set)
            sdt = fsp.tile([1, BPC], F32, tag="sdt")
            nc.scalar.activation(sdt[:], st[:, 3 * BPC:4 * BPC], AF.Ln)
            rsd = fsp.tile([1, BPC], F32, tag="rsd")
            nc.scalar.activation(rsd[:], sdt[:], AF.Exp, scale=-0.5)
            bcm = fpp.tile([DM, BPC], F32, tag="bcm")
            nc.tensor.matmul(bcm[:], ones_p[:], st[:, 0:BPC], start=True,
                             stop=True)
            bcr = fpp.tile([DM, BPC], F32, tag="bcr")
            nc.tensor.matmul(bcr[:], ones_p[:], rsd[:], start=True, stop=True)
            fc = fsp.tile([DM, BPC], F32, tag="fc")
            nc.vector.tensor_sub(fc[:], featn[:], bcm[:])
            fcn = fsp.tile([DM + 1, BPC], F32R, tag="fcn")
            nc.vector.tensor_copy(fcn[:], ones_fc[:])
            nc.vector.tensor_mul(fcn[0:DM, :], fc[:], bcr[:])
            fo = fpp.tile([BPC, NCLS], F32, tag="fo")
            nc.tensor.matmul(fo[:], fcn[:], wo_sb[:], start=True, stop=True)
            osb = fsp.tile([BPC, NCLS], F32, tag="osb")
            nc.vector.tensor_copy(osb[:], fo[:])
            nc.sync.dma_start(out[:], osb[:])


_NC_CACHE = None


def _get_program():
    global _NC_CACHE
    if _NC_CACHE is None:
        _NC_CACHE = _build_program()
    return _NC_CACHE


def _prep_inputs(x, conv_w, bn_gamma, bn_beta, Wq, bq, Wk, bk, Wv, bv, Wm, bm,
                 Wo, bo):
    f32 = np.float32
    x = np.asarray(x, f32)
    xpad = np.zeros((B, C_IN, LP), f32)
    xpad[:, :, PAD:PAD + L] = np.transpose(x, (0, 2, 1))
    wc = np.ascontiguousarray(
        np.transpose(np.asarray(conv_w, f32), (1, 2, 0)).reshape(C_IN * KW, F))

    # exact batch statistics of the conv output, on the host (fp64): the
    # column-sum and Gram matrix of the full-batch im2col reduce the
    # [B*L]-point mean/var to 52-dim quadratic forms.
    win = np.lib.stride_tricks.sliding_window_view(xpad, KW, axis=2)  # B,C,L,K
    im_all = np.ascontiguousarray(
        win.transpose(0, 1, 3, 2).reshape(B, C_IN * KW, L))
    # chunk-major contiguous layout: each [52, 500] chunk is one cheap DMA
    imcm = np.ascontiguousarray(
        im_all.reshape(B, C_IN * KW, 4, NOUT).transpose(0, 2, 1, 3))
    a = im_all.transpose(1, 0, 2).reshape(C_IN * KW, B * L).astype(np.float64)
    imsum = a.sum(axis=1)  # [52]
    gram = a @ a.T         # [52, 52]
    wc64 = wc.astype(np.float64)
    mu = (imsum @ wc64) / BN_N                            # [256]
    var = np.einsum("kf,kj,jf->f", wc64, gram, wc64) / BN_N - mu * mu
    scl = np.asarray(bn_gamma, np.float64) / np.sqrt(var + 1e-5)
    bia = np.asarray(bn_beta, np.float64) - mu * scl
    # BN scale folds into the conv weights (scale > 0 commutes with the
    # maxpool that runs before the bias+ReLU on device)
    wc_scaled = (wc64 * scl[None, :]).astype(f32)
    bia2 = np.ascontiguousarray(bia.astype(f32).reshape(2, 128).T)

    def pair_layout(W):  # [H, F, DH] -> [128, kc*512 + pr*128 + (64h0|64h1)]
        W = np.asarray(W, f32).reshape(4, 2, KC, 128, DH)  # pr, hp, kc, p, d
        o = np.transpose(W, (3, 2, 0, 1, 4))  # p, kc, pr, hp, d
        return np.ascontiguousarray(o.reshape(128, KC * 4 * 128))

    wq2, wk2 = pair_layout(Wq), pair_layout(Wk)
    wvh = np.ascontiguousarray(
        np.transpose(np.asarray(Wv, f32).reshape(H, KC, 128, DH),
                     (2, 1, 0, 3)).reshape(128, KC * H * DH)) * f32(VSCALE)
    wmh = np.ascontiguousarray(
        np.transpose(np.asarray(Wm, f32).reshape(4, 128, DM), (1, 0, 2))
        .reshape(128, 4 * DM))
    woh = np.concatenate([np.asarray(Wo, f32),
                          np.asarray(bo, f32)[None, :]], axis=0)
    # pair-layout biases: col 2pr = [bq[2pr]; bq[2pr+1]], col 2pr+1 for bk
    bq_, bk_ = np.asarray(bq, f32), np.asarray(bk, f32)
    bqkh = np.zeros((128, H), f32)
    for pr in range(4):
        bqkh[0:64, 2 * pr] = bq_[2 * pr]
        bqkh[64:128, 2 * pr] = bq_[2 * pr + 1]
        bqkh[0:64, 2 * pr + 1] = bk_[2 * pr]
        bqkh[64:128, 2 * pr + 1] = bk_[2 * pr + 1]
    bv_f = np.asarray(bv, f32).reshape(H * DH)
    bmh = np.asarray(bm, f32) + bv_f @ np.asarray(Wm, f32)

    # odd-head const layouts: [16 zero pad; 48 factor rows]
    svd_o = np.zeros((64, NOUT), f32)
    svd_o[16:64] = _SVD_WF[0:RNK]
    svd_ou = np.zeros((64, NOUT), f32)
    svd_ou[16:64] = _SVD_UT[0:RNK]

    shared = dict(wc=wc_scaled, wq2=wq2, wk2=wk2, wv=wvh, wm=wmh, wo=woh,
                  bqk=bqkh, bm_eff=bmh, bia2=bia2,
                  svd_e=_SVD_WF, svd_eu=_SVD_UT, svd_o=svd_o, svd_ou=svd_ou)
    in_maps = []
    for c in range(N_CORES):
        m = dict(shared)
        m["imcm"] = np.ascontiguousarray(imcm[c * BPC:(c + 1) * BPC])
        in_maps.append(m)
    return in_maps


def kernel(**inputs):
    in_maps = _prep_inputs(**inputs)
    nc = _get_program()
    res = bass_utils.run_bass_kernel_spmd(nc, in_maps, list(range(N_CORES)))
    return np.concatenate([res.results[i]["out"] for i in range(N_CORES)],
                          axis=0).astype(np.float32)


# revision 47
# speedup vs baseline: 408.4936x; 1.0011x over previous
"""Trainium2 Bass kernel for AttentionNet (conv frontend + MHA + readout).

Strategy: pure data-parallel over batch (64 samples -> 8 cores x 8). BatchNorm
batch statistics are computed exactly on the host from the im2col column-sum /
Gram aggregates (the affine scale folds into the conv weights, which commutes
with maxpool+ReLU because scale > 0), so there is no on-device collective and
no BN-stats prologue at all. Matmuls are fp32r except attn@V, which runs as
fp8e4 DoubleRow (contraction pairs fused; exp range-shift and a compensated
16x V pre-scale keep the quantization benign).

Per-core pipeline, software-pipelined one head ahead and across sample
boundaries (conv chunks of the next sample woven into the head loop):
  conv1d (im2col matmul from host-prepacked contiguous chunk-major layout)
  -> maxpool(4) -> fused bias+ReLU (ACT)
  -> head-pair QKV projections (M=128)
  -> scores^T with the relative-position bias folded in as a rank-48
     factorization carried in constant contraction rows (even heads keep
     data in partitions 0:64 + consts 64:112; odd heads consts 16:64 + data
     64:128, so PSUM evictions never shift partitions)
  -> exp(s/8 - 1) per 125x1000 two-bank PSUM tile into fp8 pair tiles
  -> unnormalized attn @ [V | ones] as one DoubleRow matmul per half
     (row-sums duplicated across 64 partitions = free reciprocal broadcast)
  -> normalize (with the 1/16 V-scale compensation) -> Wm + ReLU with
  accum_out readout-sum -> standardize (rsqrt via exp(-ln/2)) -> Wo.
"""

import sys

for p in ("/opt/trn_rl_repo", "/opt/pypackages"):
    if p not in sys.path:
        sys.path.insert(0, p)

import numpy as np

import concourse.bass as bass
import concourse.bacc as bacc
import concourse.tile as tile
import concourse.mybir as mybir
from concourse import bass_utils

F32 = mybir.dt.float32
F32R = mybir.dt.float32r
FP8 = mybir.dt.float8e4
DR = mybir.MatmulPerfMode.DoubleRow
AF = mybir.ActivationFunctionType
AX = mybir.AxisListType
ALU = mybir.AluOpType
ESHIFT = 1.0  # exp(x/8 - ESHIFT): range shift for fp8 attn weights
VSCALE = 16.0  # V pre-scale lifting small values out of fp8 subnormals

N_CORES = 8
B, L, C_IN = 64, 2000, 4
F, KW, PAD = 256, 13, 6
POOL = 4
H, DH = 8, 64
DM, NCLS = 100, 2
NOUT = L // POOL  # 500
BPC = B // N_CORES  # 8 samples per core
LP = L + 2 * PAD  # 2012
KC = F // 128  # 2 contraction chunks of 128
MSZ = 125  # NOUT split into 4 partition chunks of 125
RNK = 48  # rank of the bias factorization folded into the scores matmul
BN_N = float(B * L)  # batchnorm reduction count
# conv chunks of sample s+1 woven at each head of sample s
_WEAVE = {1: (0,), 2: (1,), 3: (2,), 4: (3,), 5: (4, 5), 6: (6, 7), 7: ()}


def _svd_bias_factors():
    """Rank-RNK factorization of -8*bias (added to raw qk before the 1/8
    exp scale). scoresT += ut.T @ wf; each returned block is [64, NOUT] with
    rows RNK:64 zero."""
    idx = np.arange(NOUT)
    target = -8.0 * (4.0 * np.abs(idx[:, None] - idx[None, :]) + 3.0) / (L - 1)
    U, S, Vt = np.linalg.svd(target)
    uf = (U[:, :RNK] * np.sqrt(S[:RNK])).astype(np.float32)  # [NOUT, RNK]
    wf = (np.sqrt(S[:RNK])[:, None] * Vt[:RNK]).astype(np.float32)
    ut = np.zeros((64, NOUT), np.float32)
    wz = np.zeros((64, NOUT), np.float32)
    ut[:RNK] = uf.T
    wz[:RNK] = wf
    return ut, wz


_SVD_UT, _SVD_WF = _svd_bias_factors()


def _build_program():
    nc = bacc.Bacc("TRN2", target_bir_lowering=False, debug=False,
                   num_devices=N_CORES)
    dram = {}

    def din(name, shape, dt=F32R):
        dram[name] = nc.dram_tensor(name, list(shape), dt,
                                    kind="ExternalInput").ap()
        return dram[name]

    din("imcm", [BPC, 4, C_IN * KW, NOUT])  # chunk-major contiguous im2col
    din("wc", [C_IN * KW, F])         # conv lhsT (52, 256), BN scale folded
    din("wq2", [128, KC * 4 * 128])   # [p, kc*512 + pr*128 + (head-in-pair d)]
    din("wk2", [128, KC * 4 * 128])
    din("wv", [128, KC * H * DH])     # [p, kc*512 + h*64 + d]
    din("wm", [128, 4 * DM])          # [p, c*100 + j]
    din("wo", [DM + 1, NCLS])         # [Wo; bo]
    din("svd_e", [64, NOUT])          # even-head const rows: [WF; zeros]
    din("svd_eu", [64, NOUT])         # even-head const rows: [UT; zeros]
    din("svd_o", [64, NOUT])          # odd-head const rows: [zeros; WF]
    din("svd_ou", [64, NOUT])         # odd-head const rows: [zeros; UT]
    din("bqk", [128, H], F32)         # pair-layout q/k biases
    din("bm_eff", [DM], F32)          # bm + Wm^T contribution of bv
    din("bia2", [128, 2], F32)        # host-exact BN bias (beta - mu*scale)
    out = nc.dram_tensor("out", [BPC, NCLS], F32, kind="ExternalOutput").ap()

    with tile.TileContext(nc) as tc:
        _emit(tc, dram, out)
    nc.compile()
    return nc


def _emit(tc, d, out):
    nc = tc.nc
    from contextlib import ExitStack

    ctx = ExitStack()
    with ctx:
        cst = ctx.enter_context(tc.tile_pool(name="cst", bufs=1))

        # ---- constants into SBUF ----
        bia_sb = cst.tile([128, 2], F32)
        nc.sync.dma_start(bia_sb[:], d["bia2"][:])  # tiny: warms the DMA ring
        wc_sb = cst.tile([C_IN * KW, F], F32R)
        nc.sync.dma_start(wc_sb[:], d["wc"][:])

        featn = cst.tile([DM, BPC], F32)
        ones_k = cst.tile([DM, 1], F32)
        nc.gpsimd.memset(ones_k[:], 1.0)
        ones_p = cst.tile([1, DM], F32)
        nc.gpsimd.memset(ones_p[:], 1.0)
        ones_fc = cst.tile([DM + 1, BPC], F32)
        nc.gpsimd.memset(ones_fc[:], 1.0)
        nsh = cst.tile([128, 1], F32)
        nc.gpsimd.memset(nsh[:], -ESHIFT)
        # persistent fp8 [V | ones] tiles for the DoubleRow attn@V matmuls:
        # 2 alternating sets of 2 pair-tiles [125, mcip*1024 + h*128 + d];
        # the ones regions are written once here, only V data is refreshed
        vts_sets = []
        for st in range(2):
            vset = []
            for pair in range(2):
                vt = cst.tile([MSZ, 2 * H * 128], FP8, name=f"vt{st}_{pair}")
                nc.gpsimd.memset(vt[:], 1.0)
                vset.append(vt)
            vts_sets.append(vset)

        with tc.tile_pool(name="imp", bufs=4) as imp, \
             tc.tile_pool(name="ymp", bufs=8) as ymp, \
             tc.tile_pool(name="htp", bufs=4) as htp, \
             tc.tile_pool(name="ptp", bufs=4) as ptp, \
             tc.tile_pool(name="rbp", bufs=3) as rbp, \
             tc.tile_pool(name="ocp", bufs=8) as ocp, \
             tc.tile_pool(name="msp", bufs=2) as msp, \
             tc.tile_pool(name="qkp", bufs=2, space="PSUM") as qkp, \
             tc.tile_pool(name="scp", bufs=2, space="PSUM") as scp, \
             tc.tile_pool(name="opp", bufs=2, space="PSUM") as opp:
            # ---- conv emission helpers (woven into the attention loop) ----
            imts, yms = [None] * BPC, [[None, None] for _ in range(BPC)]
            hts = [None] * BPC

            def conv_dma(s):
                # chunk-major contiguous im2col: one descriptor-light DMA per
                # 500-column chunk, each its own tile so the first conv matmul
                # only waits on its own chunk's DMA
                tiles = []
                for c in range(4):
                    t = imp.tile([C_IN * KW, NOUT], F32R, tag="imt",
                                 name="imt")
                    nc.sync.dma_start(t[:], d["imcm"][s][c])
                    tiles.append(t)
                imts[s] = tiles

            def conv_chunk(s, idx, pool=None):
                fh, c = idx // 4, idx % 4
                if yms[s][fh] is None:
                    yms[s][fh] = ymp.tile([128, NOUT], F32, tag="ym", name="ym")
                pl, tg = pool if pool is not None else (qkp, "pqk")
                ps = pl.tile([128, NOUT], F32, tag=tg, name="cvp")
                nc.tensor.matmul(
                    ps[:], wc_sb[:, fh * 128:(fh + 1) * 128],
                    imts[s][c][:], start=True, stop=True)
                nc.vector.reduce_max(
                    yms[s][fh][:, c * 125:(c + 1) * 125],
                    ps[:].rearrange("p (a b) -> p a b", b=POOL), axis=AX.X)

            def bn_relu(s):
                # pooled conv -> fused bias + ReLU on ACT (BN scale is folded
                # into the conv weights host-side)
                ht = []
                for fh in range(2):
                    t = htp.tile([128, NOUT], F32R, tag="ht", name="ht")
                    nc.scalar.activation(t[:], yms[s][fh][:], AF.Relu,
                                         bias=bia_sb[:, fh:fh + 1])
                    ht.append(t)
                hts[s] = ht

            # weight/const DMAs spread across queues so the sample-0 im2col
            # stream on the sync queue isn't head-of-line blocked
            wv_sb = cst.tile([128, KC * H * DH], F32R)
            nc.scalar.dma_start(wv_sb[:], d["wv"][:])
            wq_sb = cst.tile([128, KC * 4 * 128], F32R)
            nc.scalar.dma_start(wq_sb[:], d["wq2"][:])
            wk_sb = cst.tile([128, KC * 4 * 128], F32R)
            nc.scalar.dma_start(wk_sb[:], d["wk2"][:])
            wm_sb = cst.tile([128, 4 * DM], F32R)
            nc.scalar.dma_start(wm_sb[:], d["wm"][:])
            wo_sb = cst.tile([DM + 1, NCLS], F32R)
            nc.scalar.dma_start(wo_sb[:], d["wo"][:])
            bm_sb = cst.tile([DM, 1], F32)
            nc.scalar.dma_start(bm_sb[:], d["bm_eff"][:])

            # prologue: sample 0 conv (column-major so each im2col chunk DMA
            # feeds two matmuls as soon as it lands)
            conv_dma(0)
            bqk_sb = cst.tile([128, H], F32)
            nc.sync.dma_start(bqk_sb[:], d["bqk"][:])
            qte, qto, kte, kto = [], [], [], []
            for i in range(2):
                te = cst.tile([128, NOUT], F32R, name=f"qte{i}")
                nc.sync.dma_start(te[64:128, :], d["svd_e"][:])
                qte.append(te)
                to = cst.tile([128, NOUT], F32R, name=f"qto{i}")
                nc.sync.dma_start(to[0:64, :], d["svd_o"][:])
                qto.append(to)
                ke = cst.tile([128, NOUT], F32R, name=f"kte{i}")
                nc.sync.dma_start(ke[64:128, :], d["svd_eu"][:])
                kte.append(ke)
                ko = cst.tile([128, NOUT], F32R, name=f"kto{i}")
                nc.sync.dma_start(ko[0:64, :], d["svd_ou"][:])
                kto.append(ko)
            _pro = [(qkp, "pqk"), (opp, "op")]
            for i, idx in enumerate((0, 4, 1, 5, 2, 6, 3, 7)):
                conv_chunk(0, idx, _pro[i % 2])
            bn_relu(0)
            conv_dma(1)

            # ---- main loop: attention, software-pipelined ACROSS samples ----
            ocss = [None] * BPC

            # scores+exp for head h are emitted a head ahead of its attn@V,
            # so the exp latency hides under the next head's score matmuls;
            # the last head's attn@V drains inside the NEXT sample
            def emit_sc(s, h):
                bb = (h // 2) % 2
                qt_t = qte[bb] if h % 2 == 0 else qto[bb]
                kt_t = kte[bb] if h % 2 == 0 else kto[bb]
                op = opp.tile([128, NOUT], F32, tag="op", name="op",
                              padded_shape=[128, 512])
                pts = []
                for half in range(2):
                    sct = scp.tile([MSZ, 1024], F32, tag="sc",
                                   name="sct", padded_shape=[128, 1024])
                    for j in range(2):
                        m0 = (half * 2 + j) * MSZ
                        nc.tensor.matmul(
                            sct[:, j * 512:j * 512 + NOUT],
                            kt_t[:, m0:m0 + MSZ], qt_t[:],
                            start=True, stop=True)
                    pt = ptp.tile([MSZ, 1024], FP8, tag="pt", name="pt")
                    nc.scalar.activation(
                        pt[:].rearrange("p (b x) -> p b x", b=2)[:, :, 0:NOUT],
                        sct[:].rearrange("p (b x) -> p b x", b=2)[:, :, 0:NOUT],
                        AF.Exp, scale=1.0 / 8.0, bias=nsh[0:MSZ, :])
                    pts.append(pt)
                return (s, h, op, pts)

            def emit_av(st):
                s_, h, op, pts = st
                vv = vts_sets[s_ % 2]
                for half in range(2):
                    nc.tensor.matmul(
                        op[:, 0:NOUT],
                        vv[half][:].rearrange(
                            "p (b x) -> p b x", b=2)[:, :, h * 128:(h + 1) * 128],
                        pts[half][:].rearrange(
                            "p (b x) -> p b x", b=2)[:, :, 0:NOUT],
                        start=(half == 0), stop=(half == 1),
                        perf_mode=DR)
                rb = rbp.tile([128, NOUT], F32, tag="rb", name="rb")
                nc.vector.reciprocal_approx_fast(rb[:], op[:, 0:NOUT])
                nc.vector.scalar_tensor_tensor(
                    ocss[s_][h // 2][(h % 2) * 64:(h % 2) * 64 + 64, :],
                    op[0:64, 0:NOUT], 1.0 / VSCALE, rb[64:128, :],
                    op0=ALU.mult, op1=ALU.mult)

            def finish_sample(sp):
                mp = scp.tile([DM, NOUT], F32, tag="sc", name="mp",
                              padded_shape=[128, 512])
                for c in range(4):
                    nc.tensor.matmul(mp[:], wm_sb[:, c * DM:(c + 1) * DM],
                                     ocss[sp][c][:], start=(c == 0),
                                     stop=(c == 3))
                ms = msp.tile([DM, NOUT], F32)
                nc.scalar.activation(ms[:], mp[:], AF.Relu, bias=bm_sb[:],
                                     accum_out=featn[:, sp:sp + 1])

            pend = None
            for s in range(BPC):
                if s + 2 < BPC:
                    conv_dma(s + 2)
                ht = hts[s]
                vts = vts_sets[s % 2]
                ocss[s] = [ocp.tile([128, NOUT], F32R, tag="oc", name="oc")
                           for _ in range(4)]

                # V for all heads -> fp8 pair tiles [m, (mcip, 64 v|64 ones)];
                # vp lives in the qkp banks (free until this sample's pq)
                for mc in range(4):
                    vp = qkp.tile([MSZ, 512], F32, tag="pqk", name="vp",
                                  padded_shape=[128, 512])
                    m0 = mc * MSZ
                    for kc in range(KC):
                        nc.tensor.matmul(
                            vp[:, 0:H * DH], ht[kc][:, m0:m0 + MSZ],
                            wv_sb[:, kc * 512:(kc + 1) * 512],
                            start=(kc == 0), stop=(kc == KC - 1))
                    dst = vts[mc // 2][:, (mc % 2) * 1024:(mc % 2) * 1024 + 1024]
                    nc.vector.tensor_copy(
                        dst.rearrange("p (h x) -> p h x", x=128)[:, :, 0:DH],
                        vp[:, 0:H * DH].rearrange("p (h x) -> p h x", x=DH))

                for pr in range(4):
                    bb = pr % 2
                    pq = qkp.tile([128, NOUT], F32, tag="pqk", name="pq")
                    for kc in range(KC):
                        nc.tensor.matmul(
                            pq[:], wq_sb[:, kc * 512 + pr * 128:
                                         kc * 512 + (pr + 1) * 128],
                            ht[kc][:], start=(kc == 0), stop=(kc == KC - 1))
                    # evictions: even head -> rows 0:64, odd -> rows 64:128
                    nc.scalar.activation(
                        qte[bb][0:64, :], pq[0:64, :], AF.Identity,
                        bias=bqk_sb[0:64, 2 * pr:2 * pr + 1])
                    nc.vector.tensor_scalar_add(
                        qto[bb][64:128, :], pq[64:128, :],
                        bqk_sb[64:128, 2 * pr:2 * pr + 1])
                    pk = qkp.tile([128, NOUT], F32, tag="pqk", name="pk")
                    for kc in range(KC):
                        nc.tensor.matmul(
                            pk[:], wk_sb[:, kc * 512 + pr * 128:
                                         kc * 512 + (pr + 1) * 128],
                            ht[kc][:], start=(kc == 0), stop=(kc == KC - 1))
                    nc.vector.tensor_scalar_add(
                        kte[bb][0:64, :], pk[0:64, :],
                        bqk_sb[0:64, 2 * pr + 1:2 * pr + 2])
                    nc.scalar.activation(
                        kto[bb][0 + 64:128, :], pk[64:128, :], AF.Identity,
                        bias=bqk_sb[64:128, 2 * pr + 1:2 * pr + 2])

                    for h in (2 * pr, 2 * pr + 1):
                        st = emit_sc(s, h)
                        if pend is not None:
                            emit_av(pend)
                            if h == 0:
                                finish_sample(s - 1)
                            # weave next sample's conv so its pool + bias+ReLU
                            # clear the DVE/ACT queues well before the boundary
                            if s + 1 < BPC:
                                for cidx in _WEAVE.get(h, ()):
                                    conv_chunk(s + 1, cidx)
                                if h == 6:
                                    bn_relu(s + 1)
                        pend = st
            emit_av(pend)
            finish_sample(BPC - 1)

        # ---- final: standardize + Wo ----
        with tc.tile_pool(name="fsp", bufs=1) as fsp, \
             tc.tile_pool(name="fpp", bufs=1, space="PSUM") as fpp:
            fsq = fsp.tile([DM, BPC], F32)
            nc.vector.tensor_mul(fsq[:], featn[:], featn[:])
            cs = fpp.tile([1, 2 * BPC], F32, tag="cs")
            nc.tensor.matmul(cs[:, 0:BPC], ones_k[:], featn[:], start=True,
                             stop=True)
            nc.tensor.matmul(cs[:, BPC:2 * BPC], ones_k[:], fsq[:], start=True,
                             stop=True)
            st = fsp.tile([1, 4 * BPC], F32)
            nc.vector.tensor_scalar_mul(st[:, 0:BPC], cs[:, 0:BPC], 1.0 / DM)
            nc.vector.tensor_scalar_mul(st[:, BPC:2 * BPC], cs[:, BPC:2 * BPC],
                                        1.0 / DM)
            nc.vector.tensor_mul(st[:, 2 * BPC:3 * BPC], st[:, 0:BPC],
                                 st[:, 0:BPC])
            nc.vector.tensor_sub(st[:, 3 * BPC:4 * BPC], st[:, BPC:2 * BPC],
                                 st[:, 2 * BPC:3 * BPC])
            # 1/(sqrt(v)+1e-6) ~= exp(-0.5*ln(v)) (v ~ O(1e4+), eps negligible;
            # ln+exp share the already-loaded natural_log_exp table set)
            sdt = fsp.tile([1, BPC], F32, tag="sdt")
            nc.scalar.activation(sdt[:], st[:, 3 * BPC:4 * BPC], AF.Ln)
            rsd = fsp.tile([1, BPC], F32, tag="rsd")
            nc.scalar.activation(rsd[:], sdt[:], AF.Exp, scale=-0.5)
            bcm = fpp.tile([DM, BPC], F32, tag="bcm")
            nc.tensor.matmul(bcm[:], ones_p[:], st[:, 0:BPC], start=True,
                             stop=True)
            bcr = fpp.tile([DM, BPC], F32, tag="bcr")
            nc.tensor.matmul(bcr[:], ones_p[:], rsd[:], start=True, stop=True)
            fc = fsp.tile([DM, BPC], F32, tag="fc")
            nc.vector.tensor_sub(fc[:], featn[:], bcm[:])
            fcn = fsp.tile([DM + 1, BPC], F32R, tag="fcn")
            nc.vector.tensor_copy(fcn[:], ones_fc[:])
            nc.vector.tensor_mul(fcn[0:DM, :], fc[:], bcr[:])
            fo = fpp.tile([BPC, NCLS], F32, tag="fo")
            nc.tensor.matmul(fo[:], fcn[:], wo_sb[:], start=True, stop=True)
            osb = fsp.tile([BPC, NCLS], F32, tag="osb")
            nc.vector.tensor_copy(osb[:], fo[:])
            nc.sync.dma_start(out[:], osb[:])


_NC_CACHE = None


def _get_program():
    global _NC_CACHE
    if _NC_CACHE is None:
        _NC_CACHE = _build_program()
    return _NC_CACHE


def _prep_inputs(x, conv_w, bn_gamma, bn_beta, Wq, bq, Wk, bk, Wv, bv, Wm, bm,
                 Wo, bo):
    f32 = np.float32
    x = np.asarray(x, f32)
    xpad = np.zeros((B, C_IN, LP), f32)
    xpad[:, :, PAD:PAD + L] = np.transpose(x, (0, 2, 1))
    wc = np.ascontiguousarray(
        np.transpose(np.asarray(conv_w, f32), (1, 2, 0)).reshape(C_IN * KW, F))

    # exact batch statistics of the conv output, on the host (fp64): the
    # column-sum and Gram matrix of the full-batch im2col reduce the
    # [B*L]-point mean/var to 52-dim quadratic forms.
    win = np.lib.stride_tricks.sliding_window_view(xpad, KW, axis=2)  # B,C,L,K
    im_all = np.ascontiguousarray(
        win.transpose(0, 1, 3, 2).reshape(B, C_IN * KW, L))
    # chunk-major contiguous layout: each [52, 500] chunk is one cheap DMA
    imcm = np.ascontiguousarray(
        im_all.reshape(B, C_IN * KW, 4, NOUT).transpose(0, 2, 1, 3))
    a = im_all.transpose(1, 0, 2).reshape(C_IN * KW, B * L).astype(np.float64)
    imsum = a.sum(axis=1)  # [52]
    gram = a @ a.T         # [52, 52]
    wc64 = wc.astype(np.float64)
    mu = (imsum @ wc64) / BN_N                            # [256]
    var = np.einsum("kf,kj,jf->f", wc64, gram, wc64) / BN_N - mu * mu
    scl = np.asarray(bn_gamma, np.float64) / np.sqrt(var + 1e-5)
    bia = np.asarray(bn_beta, np.float64) - mu * scl
    # BN scale folds into the conv weights (scale > 0 commutes with the
    # maxpool that runs before the bias+ReLU on device)
    wc_scaled = (wc64 * scl[None, :]).astype(f32)
    bia2 = np.ascontiguousarray(bia.astype(f32).reshape(2, 128).T)

    def pair_layout(W):  # [H, F, DH] -> [128, kc*512 + pr*128 + (64h0|64h1)]
        W = np.asarray(W, f32).reshape(4, 2, KC, 128, DH)  # pr, hp, kc, p, d
        o = np.transpose(W, (3, 2, 0, 1, 4))  # p, kc, pr, hp, d
        return np.ascontiguousarray(o.reshape(128, KC * 4 * 128))

    wq2, wk2 = pair_layout(Wq), pair_layout(Wk)
    wvh = np.ascontiguousarray(
        np.transpose(np.asarray(Wv, f32).reshape(H, KC, 128, DH),
                     (2, 1, 0, 3)).reshape(128, KC * H * DH)) * f32(VSCALE)
    wmh = np.ascontiguousarray(
        np.transpose(np.asarray(Wm, f32).reshape(4, 128, DM), (1, 0, 2))
        .reshape(128, 4 * DM))
    woh = np.concatenate([np.asarray(Wo, f32),
                          np.asarray(bo, f32)[None, :]], axis=0)
    # pair-layout biases: col 2pr = [bq[2pr]; bq[2pr+1]], col 2pr+1 for bk
    bq_, bk_ = np.asarray(bq, f32), np.asarray(bk, f32)
    bqkh = np.zeros((128, H), f32)
    for pr in range(4):
        bqkh[0:64, 2 * pr] = bq_[2 * pr]
        bqkh[64:128, 2 * pr] = bq_[2 * pr + 1]
        bqkh[0:64, 2 * pr + 1] = bk_[2 * pr]
        bqkh[64:128, 2 * pr + 1] = bk_[2 * pr + 1]
    bv_f = np.asarray(bv, f32).reshape(H * DH)
    bmh = np.asarray(bm, f32) + bv_f @ np.asarray(Wm, f32)

    # odd-head const layouts: [16 zero pad; 48 factor rows]
    svd_o = np.zeros((64, NOUT), f32)
    svd_o[16:64] = _SVD_WF[0:RNK]
    svd_ou = np.zeros((64, NOUT), f32)
    svd_ou[16:64] = _SVD_UT[0:RNK]

    shared = dict(wc=wc_scaled, wq2=wq2, wk2=wk2, wv=wvh, wm=wmh, wo=woh,
                  bqk=bqkh, bm_eff=bmh, bia2=bia2,
                  svd_e=_SVD_WF, svd_eu=_SVD_UT, svd_o=svd_o, svd_ou=svd_ou)
    in_maps = []
    for c in range(N_CORES):
        m = dict(shared)
        m["imcm"] = np.ascontiguousarray(imcm[c * BPC:(c + 1) * BPC])
        in_maps.append(m)
    return in_maps


def kernel(**inputs):
    in_maps = _prep_inputs(**inputs)
    nc = _get_program()
    res = bass_utils.run_bass_kernel_spmd(nc, in_maps, list(range(N_CORES)))
    return np.concatenate([res.results[i]["out"] for i in range(N_CORES)],
                          axis=0).astype(np.float32)
